# revision 1
# baseline (speedup 1.0000x reference)
"""DGCNN Bass kernel for TRN2 — one sample per NeuronCore.

Math (per graph-conv layer, BN folded on host):
  u[n,m] = 2<x_n,x_m> - |x_m|^2   (row-wise top-20 == reference kNN)
  As = X^T (s*W1)^T [N,O];  Bs = X^T (s*(W2-W1))^T + t [N,O]
  h[n,o] = LReLU(max_{j in knn(n)} As[j,o] + Bs[n,o]);  X_next = h^T
Tail: conv1d(512->1024)+BN+LReLU, global max/avg pool, FC 2048->512->256->40.
"""
import numpy as np

import concourse.bass as bass
import concourse.bacc as bacc
import concourse.mybir as mybir
from concourse.tile import TileContext
from concourse.masks import make_identity

FP = mybir.dt.float32
AF = mybir.ActivationFunctionType
ALU = mybir.AluOpType
N = 2048
K = 20
NEG = 0.2
NT = N // 128  # 16 row tiles

LAYERS = [(3, 64), (64, 64), (64, 128), (128, 256)]  # (C_in, O)


def host_prep(inputs, core):
    """Per-core parameter dict from the full input dict (numpy arrays)."""
    d = {k: np.asarray(v, np.float32) for k, v in inputs.items()}
    p = {"x": np.ascontiguousarray(d["x"][core])}  # [3, N]
    for li, (c, o) in enumerate(LAYERS):
        w = d[f"w_gc{li}"]                      # [O, 2C]
        s = d[f"s{li}"]
        t = d[f"t{li}"]
        w1 = w[:, :c] * s[:, None]              # [O, C]
        w2 = (w[:, c:] - w[:, :c]) * s[:, None]
        p[f"w1_{li}"] = np.ascontiguousarray(w1.T)     # [C, O]
        p[f"w2_{li}"] = np.ascontiguousarray(w2.T)
        p[f"t_{li}"] = np.ascontiguousarray(
            np.broadcast_to(t[None, :], (128, o)))     # replicated bias
    p["w_c1d_T"] = np.ascontiguousarray((d["w_c1d"] * d["s4"][:, None]).T)  # [512,1024]
    p["t4"] = np.ascontiguousarray(d["t4"].reshape(8, 128).T)  # [128, 8] per-mg cols
    wl1 = d["w_l1"] * d["s5"][:, None]          # [512, 2048]
    wl1 = wl1.copy()
    wl1[:, 1024:] /= float(N)                   # fold avg-pool divisor
    p["w_l1_T"] = np.ascontiguousarray(wl1.T)   # [2048, 512]
    p["t5"] = np.ascontiguousarray(d["t5"][None, :])   # [1, 512]
    wl2 = d["w_l2"] * d["s6"][:, None]
    p["w_l2_T"] = np.ascontiguousarray(wl2.T)   # [512, 256]
    p["t6"] = np.ascontiguousarray(
        (d["s6"] * d["b_l2"] + d["t6"])[None, :])      # [1, 256]
    p["w_l3_T"] = np.ascontiguousarray(d["w_l3"].T)    # [256, 40]
    p["b_l3"] = np.ascontiguousarray(d["b_l3"][None, :])
    return p


def build(dbg=()):
    nc = bacc.Bacc(None, target_bir_lowering=False, num_swdge_queues=4)
    x_in = nc.declare_dram_parameter("x", [3, N], FP, isOutput=False)
    W1, W2, TB = {}, {}, {}
    for li, (c, o) in enumerate(LAYERS):
        W1[li] = nc.declare_dram_parameter(f"w1_{li}", [c, o], FP, isOutput=False)
        W2[li] = nc.declare_dram_parameter(f"w2_{li}", [c, o], FP, isOutput=False)
        TB[li] = nc.declare_dram_parameter(f"t_{li}", [128, o], FP, isOutput=False)
    wc = nc.declare_dram_parameter("w_c1d_T", [512, 1024], FP, isOutput=False)
    t4 = nc.declare_dram_parameter("t4", [128, 8], FP, isOutput=False)
    wl1 = nc.declare_dram_parameter("w_l1_T", [2048, 512], FP, isOutput=False)
    t5 = nc.declare_dram_parameter("t5", [1, 512], FP, isOutput=False)
    wl2 = nc.declare_dram_parameter("w_l2_T", [512, 256], FP, isOutput=False)
    t6 = nc.declare_dram_parameter("t6", [1, 256], FP, isOutput=False)
    wl3 = nc.declare_dram_parameter("w_l3_T", [256, 40], FP, isOutput=False)
    bl3 = nc.declare_dram_parameter("b_l3", [1, 40], FP, isOutput=False)
    out = nc.declare_dram_parameter("out", [1, 40], FP, isOutput=True)

    dbgt = {}
    for name in dbg:
        shp = {"u0": [128, N], "idx0": [128, 24],
               "h0": [64, N], "h1": [64, N], "h2": [128, N], "h3": [256, N],
               "z4": [128, N], "zpool": [128, 16]}[name]
        dbgt[name] = nc.declare_dram_parameter("dbg_" + name, shp, FP, isOutput=True)

    with TileContext(nc) as tc:
        with (
            tc.tile_pool(name="const", bufs=1) as cp,
            tc.tile_pool(name="dram", bufs=2, space="DRAM") as dp,
            tc.tile_pool(name="x", bufs=1) as xp,
        ):
            ident = cp.tile([128, 128], FP)
            make_identity(nc, ident)
            xl = xp.tile([128, N], FP)   # [2X; ones(row C) for C<128]
            xr = xp.tile([128, N], FP)   # [X; -sq(row C) for C<128]
            xa1 = xp.tile([1, N], FP)    # ones row (L4)
            xan = xp.tile([1, N], FP)    # -sq row (L4)
            hts = []
            for li in range(4):
                o_l = LAYERS[li][1]
                h_t = xp.tile([min(o_l, 128), (o_l // 128 or 1) * N], FP,
                              tag=f"h{li}")
                hts.append(h_t)
            nc.sync.dma_start(out=xr[0:3, :], in_=x_in[:, :])

            for li, (C, O) in enumerate(LAYERS):
                _layer(nc, tc, dp, li, C, O, xl, xr, xa1, xan, ident,
                       W1[li], W2[li], TB[li], hts[li], dbgt)

            _tail(nc, tc, hts, wc, t4, wl1, t5, wl2, t6, wl3, bl3, out, dbgt)
    nc.finalize()
    return nc


def _layer(nc, tc, dp, li, C, O, xl, xr, xa1, xan, ident, w1, w2, tbias,
           h_out, dbgt):
    Ka = C + 1
    as_dram = dp.tile([N, 256], FP, tag="as_dram")
    with (
        tc.tile_pool(name=f"L{li}", bufs=3) as pool,
        tc.tile_pool(name=f"L{li}u", bufs=2) as pu,
        tc.tile_pool(name=f"L{li}c", bufs=1) as pc,
        tc.tile_pool(name=f"L{li}g", bufs=2 if li == 3 else 3) as pg,
        tc.tile_pool(name=f"L{li}up", bufs=1, space="PSUM") as ppu,
        tc.tile_pool(name=f"L{li}sp", bufs=3, space="PSUM") as pps,
    ):
        # ---- augmented X ----
        nc.vector.tensor_scalar(xl[0:C, :], xr[0:C, :], 2.0, None, ALU.mult)
        nc.vector.memset(xa1, 1.0)
        xsq = pc.tile([C, N], FP, tag="xsq")
        nc.scalar.activation(xsq, xr[0:C, :], AF.Square)
        ones_c = pc.tile([C, 1], FP, tag="ones_c")
        nc.vector.memset(ones_c, 1.0)
        for j in range(4):
            cs = slice(j * 512, (j + 1) * 512)
            sq_ps = pps.tile([1, 512], FP, tag="sps")
            nc.tensor.matmul(sq_ps, ones_c[0:C, :], xsq[0:C, cs],
                             start=True, stop=True)
            nc.scalar.activation(xan[:, cs], sq_ps, AF.Identity, scale=-1.0)
        if C < 128:
            # place aug rows at partition C (DMA: no base-partition limits)
            nc.sync.dma_start(out=xl[C:C + 1, :], in_=xa1)
            nc.sync.dma_start(out=xr[C:C + 1, :], in_=xan)

        # ---- weights ----
        w1t = pc.tile([C, O], FP, tag="w1t")
        w2t = pc.tile([C, O], FP, tag="w2t")
        tb = pc.tile([128, O], FP, tag="tb")
        nc.sync.dma_start(out=w1t, in_=w1[:, :])
        nc.sync.dma_start(out=w2t, in_=w2[:, :])
        nc.sync.dma_start(out=tb, in_=tbias[:, :])

        bs_all = pc.tile([128, NT * O], FP, tag="bs_all")
        idx_all = pc.tile([128, NT * 24], mybir.dt.uint32, tag="idx_all")
        as_view = as_dram[:, :].rearrange("n o -> (n o)")[0:N * O]            .rearrange("(n o) -> n o", o=O)

        # ======== pass A: As/Bs + u + topk + idx ========
        for i in range(NT):
            nl = slice(i * 128, (i + 1) * 128)
            asps = pps.tile([128, O], FP, tag="sps")
            nc.tensor.matmul(asps, xr[0:C, nl], w1t[0:C, :], start=True, stop=True)
            as_sb = pool.tile([128, O], FP, tag="as_sb")
            nc.scalar.activation(as_sb, asps, AF.Identity)
            nc.sync.dma_start(out=as_view[nl, :], in_=as_sb)
            bsps = pps.tile([128, O], FP, tag="sps")
            nc.tensor.matmul(bsps, xr[0:C, nl], w2t[0:C, :], start=True, stop=True)
            nc.vector.tensor_tensor(bs_all[:, i * O:(i + 1) * O], bsps, tb, ALU.add)

            ups = ppu.tile([128, N], FP, tag="ups")
            for j in range(4):
                cs = slice(j * 512, (j + 1) * 512)
                if Ka <= 128:
                    nc.tensor.matmul(ups[:, cs], xl[0:Ka, nl], xr[0:Ka, cs],
                                     start=True, stop=True)
                else:
                    nc.tensor.matmul(ups[:, cs], xl[0:C, nl], xr[0:C, cs],
                                     start=True, stop=False)
                    nc.tensor.matmul(ups[:, cs], xa1[:, nl], xan[:, cs],
                                     start=False, stop=True)
            u = pool.tile([128, N], FP, tag="u")
            nc.scalar.activation(u, ups, AF.Identity)
            if "u0" in dbgt and li == 0 and i == 0:
                nc.sync.dma_start(out=dbgt["u0"][:, :], in_=u)

            vals = pool.tile([128, 24], FP, tag="vals")
            idxu = idx_all[:, i * 24:(i + 1) * 24]
            u2 = pu.tile([128, N], FP, tag="u2")
            u3 = pu.tile([128, N], FP, tag="u3")
            nc.vector.max(out=vals[:, 0:8], in_=u)
            nc.vector.max_index(out=idxu[:, 0:8], in_max=vals[:, 0:8], in_values=u)
            nc.vector.match_replace(out=u2, in_to_replace=vals[:, 0:8],
                                    in_values=u, imm_value=-1e30)
            nc.vector.max(out=vals[:, 8:16], in_=u2)
            nc.vector.max_index(out=idxu[:, 8:16], in_max=vals[:, 8:16],
                                in_values=u2)
            nc.vector.match_replace(out=u3, in_to_replace=vals[:, 8:16],
                                    in_values=u2, imm_value=-1e30)
            nc.vector.max(out=vals[:, 16:24], in_=u3)
            nc.vector.max_index(out=idxu[:, 16:24], in_max=vals[:, 16:24],
                                in_values=u3)
            if "idx0" in dbgt and li == 0 and i == 0:
                fi = pool.tile([128, 24], FP, tag="fi")
                nc.vector.tensor_copy(fi, idxu)
                nc.sync.dma_start(out=dbgt["idx0"][:, :], in_=fi)


        # ======== pass B: indirect-DMA gather (128 rows x 1 idx col) ========
        for i in range(NT):
            nl = slice(i * 128, (i + 1) * 128)
            gath = pg.tile([128, K, O], FP, tag="gath")
            for kk in range(K):
                nc.gpsimd.indirect_dma_start(
                    out=gath[:, kk, :], out_offset=None,
                    in_=as_view[0:N, 0:O],
                    in_offset=bass.IndirectOffsetOnAxis(
                        ap=idx_all[:, i * 24 + kk:i * 24 + kk + 1], axis=0))
            m = pool.tile([128, O], FP, tag="m")
            nc.vector.tensor_reduce(m, gath.rearrange("p k o -> p o k"),
                                    mybir.AxisListType.X, ALU.max)
            nc.vector.tensor_tensor(m, m, bs_all[:, i * O:(i + 1) * O], ALU.add)
            hl = pool.tile([128, O], FP, tag="hl")
            nc.vector.scalar_tensor_tensor(hl, m, NEG, m, ALU.mult, ALU.max)
            for ob in range((O + 127) // 128):
                osl = slice(ob * 128, min((ob + 1) * 128, O))
                ow = osl.stop - osl.start
                tps = pps.tile([128, 128], FP, tag="sps")
                nc.tensor.transpose(tps[0:ow, :], hl[:, osl], ident)
                if O > 128:
                    dst = h_out[0:ow, ob * N + i * 128:ob * N + (i + 1) * 128]
                else:
                    dst = h_out[osl, nl]
                nc.scalar.activation(dst, tps[0:ow, :], AF.Identity)

        if li < 3:
            nc.vector.tensor_copy(xr[0:O, :], h_out[0:O, :])
        if f"h{li}" in dbgt:
            if O > 128:
                nc.sync.dma_start(
                    out=dbgt[f"h{li}"][:, :].rearrange("(g p) n -> p g n", p=128),
                    in_=h_out.rearrange("p (g n) -> p g n", g=2))
            else:
                nc.sync.dma_start(out=dbgt[f"h{li}"][:, :], in_=h_out)


def _tail(nc, tc, hts, wc, t4, wl1, t5, wl2, t6, wl3, bl3, out, dbgt):
    with (
        tc.tile_pool(name="T", bufs=2) as pool,
        tc.tile_pool(name="Tc", bufs=1) as pc,
        tc.tile_pool(name="Tp", bufs=4, space="PSUM") as pps,
    ):
        # 5 K-pieces (h0, h1, h2, h3a, h3b), each loaded at base partition 0
        wct = pc.tile([128, 5 * 1024], FP, tag="wct")
        pieces = [(0, 64), (64, 128), (128, 256), (256, 384), (384, 512)]
        for pi, (r0, r1_) in enumerate(pieces):
            nc.sync.dma_start(out=wct[0:r1_ - r0, pi * 1024:(pi + 1) * 1024],
                              in_=wc[r0:r1_, :])
        t4t = pc.tile([128, 8], FP, tag="t4t")
        nc.sync.dma_start(out=t4t, in_=t4[:, :])
        # z rows: h0(64) h1(64) h2(128) h3(256): K-chunks of 128:
        # chunk0 = [h0; h1], chunk1 = h2, chunk2 = h3[0:128], chunk3 = h3[128:256]
        zmax = pc.tile([128, 8], FP, tag="zmax")
        zsum = pc.tile([128, 8], FP, tag="zsum")
        for mg in range(8):
            ml = slice(mg * 128, (mg + 1) * 128)
            z4g = pool.tile([128, N], FP, tag="z4g")
            for j in range(4):
                cs = slice(j * 512, (j + 1) * 512)
                zps = pps.tile([128, 512], FP, tag="zps")

                def wv(pi, kk):
                    return wct[0:kk, pi * 1024 + mg * 128:pi * 1024 + (mg + 1) * 128]

                nc.tensor.matmul(zps, wv(0, 64), hts[0][:, cs],
                                 start=True, stop=False)
                nc.tensor.matmul(zps, wv(1, 64), hts[1][:, cs],
                                 start=False, stop=False)
                nc.tensor.matmul(zps, wv(2, 128), hts[2][:, cs],
                                 start=False, stop=False)
                nc.tensor.matmul(zps, wv(3, 128), hts[3][:, cs],
                                 start=False, stop=False)
                nc.tensor.matmul(zps, wv(4, 128),
                                 hts[3][:, N + cs.start:N + cs.stop],
                                 start=False, stop=True)
                nc.scalar.activation(z4g[:, cs], zps, AF.Identity,
                                     bias=t4t[:, mg:mg + 1])
                nc.vector.scalar_tensor_tensor(z4g[:, cs], z4g[:, cs], NEG,
                                               z4g[:, cs], ALU.mult, ALU.max)
            if "z4" in dbgt and mg == 0:
                nc.sync.dma_start(out=dbgt["z4"][:, :], in_=z4g)
            nc.vector.tensor_reduce(zmax[:, mg:mg + 1], z4g,
                                    mybir.AxisListType.X, ALU.max)
            nc.vector.tensor_reduce(zsum[:, mg:mg + 1], z4g,
                                    mybir.AxisListType.X, ALU.add)
        if "zpool" in dbgt:
            zz = pool.tile([128, 16], FP, tag="zz")
            nc.vector.tensor_copy(zz[:, 0:8], zmax)
            nc.vector.tensor_copy(zz[:, 8:16], zsum)
            nc.sync.dma_start(out=dbgt["zpool"][:, :], in_=zz)

        # ---- FC tail (transposed matvecs, out stays [1, M]) ----
        def fc(z_cols, w_T, M, bias_row, lrelu):
            # z_cols: [128, nk] tile (K-chunks as columns); w_T: [128*nk, M] dram
            nk = z_cols.shape[1]
            ps = pps.tile([1, 512], FP, tag="fps")
            wt = pc.tile([128, nk * M], FP, tag=f"w{M}")
            nc.sync.dma_start(out=wt.rearrange("p (a o) -> p a o", o=M),
                              in_=w_T[:, :].rearrange("(a p) o -> p a o", p=128))
            for kc in range(nk):
                nc.tensor.matmul(ps[0:1, 0:M], z_cols[:, kc:kc + 1],
                                 wt[:, kc * M:(kc + 1) * M],
                                 start=(kc == 0), stop=(kc == nk - 1))
            row = pool.tile([1, M], FP, tag=f"row{M}")
            br = pool.tile([1, M], FP, tag=f"br{M}")
            nc.sync.dma_start(out=br, in_=bias_row[:, :])
            nc.scalar.activation(row, ps[0:1, 0:M], AF.Identity)
            nc.vector.tensor_tensor(row, row, br, ALU.add)
            if lrelu:
                nc.vector.scalar_tensor_tensor(row, row, NEG, row,
                                               ALU.mult, ALU.max)
            return row

        fcd = tc.tile_pool(name="fcd", bufs=2, space="DRAM")
        with fcd as fdp:
            z16 = pc.tile([128, 16], FP, tag="z16")
            nc.vector.tensor_copy(z16[:, 0:8], zmax)
            nc.vector.tensor_copy(z16[:, 8:16], zsum)
            r1 = fc(z16, wl1, 512, t5, True)                  # [1, 512]
            b1 = fdp.tile([512], FP, tag="fcb")
            nc.sync.dma_start(out=b1[:], in_=r1)
            z1c = pc.tile([128, 4], FP, tag="z1c")
            nc.sync.dma_start(out=z1c, in_=b1.rearrange("(a p) -> p a", p=128))
            r2 = fc(z1c, wl2, 256, t6, True)                  # [1, 256]
            b2 = fdp.tile([512], FP, tag="fcb")
            nc.sync.dma_start(out=b2[0:256], in_=r2)
            z2c = pc.tile([128, 2], FP, tag="z2c")
            nc.sync.dma_start(out=z2c,
                              in_=b2[0:256].rearrange("(a p) -> p a", p=128))
            r3 = fc(z2c, wl3, 40, bl3, False)                 # [1, 40]
            nc.sync.dma_start(out=out[:, :], in_=r3)


# ---------------------------------------------------------------------------
# Harness entry point: kernel(**inputs) -> np.ndarray [8, 40]
# ---------------------------------------------------------------------------
_NC_CACHE = {}


def _get_nc():
    if "nc" not in _NC_CACHE:
        _NC_CACHE["nc"] = build()
    return _NC_CACHE["nc"]


def kernel(**inputs):
    from concourse.bass_utils import run_bass_kernel_spmd

    nc = _get_nc()
    in_maps = [host_prep(inputs, core) for core in range(8)]
    res = run_bass_kernel_spmd(nc, in_maps, core_ids=list(range(8)))
    out = np.stack([r["out"][0] for r in res.results]).astype(np.float32)
    return out



# revision 4
# speedup vs baseline: 1.0337x; 1.0337x over previous
"""DGCNN Bass kernel for TRN2 — one sample per NeuronCore.

Math (per graph-conv layer, BN folded on host):
  u[n,m] = 2<x_n,x_m> - |x_m|^2   (row-wise top-20 == reference kNN)
  As = X^T (s*W1)^T [N,O];  Bs = X^T (s*(W2-W1))^T + t [N,O]
  h[n,o] = LReLU(max_{j in knn(n)} As[j,o] + Bs[n,o]);  X_next = h^T
Tail: conv1d(512->1024)+BN+LReLU, global max/avg pool, FC 2048->512->256->40.
"""
import numpy as np

import concourse.bass as bass
import concourse.bacc as bacc
import concourse.mybir as mybir
from concourse.tile import TileContext
from concourse.masks import make_identity

FP = mybir.dt.float32
AF = mybir.ActivationFunctionType
ALU = mybir.AluOpType
N = 2048
K = 20
NEG = 0.2
NT = N // 128  # 16 row tiles

LAYERS = [(3, 64), (64, 64), (64, 128), (128, 256)]  # (C_in, O)


def host_prep(inputs, core):
    """Per-core parameter dict from the full input dict (numpy arrays)."""
    d = {k: np.asarray(v, np.float32) for k, v in inputs.items()}
    p = {"x": np.ascontiguousarray(d["x"][core])}  # [3, N]
    for li, (c, o) in enumerate(LAYERS):
        w = d[f"w_gc{li}"]                      # [O, 2C]
        s = d[f"s{li}"]
        t = d[f"t{li}"]
        w1 = w[:, :c] * s[:, None]              # [O, C]
        w2 = (w[:, c:] - w[:, :c]) * s[:, None]
        p[f"w1_{li}"] = np.ascontiguousarray(w1.T)     # [C, O]
        p[f"w2_{li}"] = np.ascontiguousarray(w2.T)
        p[f"t_{li}"] = np.ascontiguousarray(
            np.broadcast_to(t[None, :], (128, o)))     # replicated bias
    p["w_c1d_T"] = np.ascontiguousarray((d["w_c1d"] * d["s4"][:, None]).T)  # [512,1024]
    p["t4"] = np.ascontiguousarray(d["t4"].reshape(8, 128).T)  # [128, 8] per-mg cols
    wl1 = d["w_l1"] * d["s5"][:, None]          # [512, 2048]
    wl1 = wl1.copy()
    wl1[:, 1024:] /= float(N)                   # fold avg-pool divisor
    p["w_l1_T"] = np.ascontiguousarray(wl1.T)   # [2048, 512]
    p["t5"] = np.ascontiguousarray(d["t5"][None, :])   # [1, 512]
    wl2 = d["w_l2"] * d["s6"][:, None]
    p["w_l2_T"] = np.ascontiguousarray(wl2.T)   # [512, 256]
    p["t6"] = np.ascontiguousarray(
        (d["s6"] * d["b_l2"] + d["t6"])[None, :])      # [1, 256]
    p["w_l3_T"] = np.ascontiguousarray(d["w_l3"].T)    # [256, 40]
    p["b_l3"] = np.ascontiguousarray(d["b_l3"][None, :])
    return p


def build(dbg=()):
    nc = bacc.Bacc(None, target_bir_lowering=False, num_swdge_queues=4)
    x_in = nc.declare_dram_parameter("x", [3, N], FP, isOutput=False)
    W1, W2, TB = {}, {}, {}
    for li, (c, o) in enumerate(LAYERS):
        W1[li] = nc.declare_dram_parameter(f"w1_{li}", [c, o], FP, isOutput=False)
        W2[li] = nc.declare_dram_parameter(f"w2_{li}", [c, o], FP, isOutput=False)
        TB[li] = nc.declare_dram_parameter(f"t_{li}", [128, o], FP, isOutput=False)
    wc = nc.declare_dram_parameter("w_c1d_T", [512, 1024], FP, isOutput=False)
    t4 = nc.declare_dram_parameter("t4", [128, 8], FP, isOutput=False)
    wl1 = nc.declare_dram_parameter("w_l1_T", [2048, 512], FP, isOutput=False)
    t5 = nc.declare_dram_parameter("t5", [1, 512], FP, isOutput=False)
    wl2 = nc.declare_dram_parameter("w_l2_T", [512, 256], FP, isOutput=False)
    t6 = nc.declare_dram_parameter("t6", [1, 256], FP, isOutput=False)
    wl3 = nc.declare_dram_parameter("w_l3_T", [256, 40], FP, isOutput=False)
    bl3 = nc.declare_dram_parameter("b_l3", [1, 40], FP, isOutput=False)
    out = nc.declare_dram_parameter("out", [1, 40], FP, isOutput=True)

    dbgt = {}
    for name in dbg:
        shp = {"u0": [128, N], "idx0": [128, 24],
               "h0": [64, N], "h1": [64, N], "h2": [128, N], "h3": [256, N],
               "z4": [128, N], "zpool": [128, 16]}[name]
        dbgt[name] = nc.declare_dram_parameter("dbg_" + name, shp, FP, isOutput=True)

    with TileContext(nc) as tc:
        with (
            tc.tile_pool(name="const", bufs=1) as cp,
            tc.tile_pool(name="dram", bufs=2, space="DRAM") as dp,
            tc.tile_pool(name="x", bufs=1) as xp,
        ):
            ident = cp.tile([128, 128], FP)
            make_identity(nc, ident)
            xl = xp.tile([128, N], FP)   # [2X; ones(row C) for C<128]
            xr = xp.tile([128, N], FP)   # [X; -sq(row C) for C<128]
            xa1 = xp.tile([1, N], FP)    # ones row (L4)
            xan = xp.tile([1, N], FP)    # -sq row (L4)
            hts = []
            for li in range(4):
                o_l = LAYERS[li][1]
                h_t = xp.tile([min(o_l, 128), (o_l // 128 or 1) * N], FP,
                              tag=f"h{li}")
                hts.append(h_t)
            nc.sync.dma_start(out=xr[0:3, :], in_=x_in[:, :])

            for li, (C, O) in enumerate(LAYERS):
                _layer(nc, tc, dp, li, C, O, xl, xr, xa1, xan, ident,
                       W1[li], W2[li], TB[li], hts[li], dbgt)

            _tail(nc, tc, hts, wc, t4, wl1, t5, wl2, t6, wl3, bl3, out, dbgt)
    nc.finalize()
    return nc


def _layer(nc, tc, dp, li, C, O, xl, xr, xa1, xan, ident, w1, w2, tbias,
           h_out, dbgt):
    Ka = C + 1
    as_dram = dp.tile([N, 256], FP, tag="as_dram")
    with (
        tc.tile_pool(name=f"L{li}", bufs=3) as pool,
        tc.tile_pool(name=f"L{li}u", bufs=1) as pu,
        tc.tile_pool(name=f"L{li}ud", bufs=2) as pud,
        tc.tile_pool(name=f"L{li}c", bufs=1) as pc,
        tc.tile_pool(name=f"L{li}g", bufs=2 if li >= 2 else 3) as pg,
        tc.tile_pool(name=f"L{li}up", bufs=1, space="PSUM") as ppu,
        tc.tile_pool(name=f"L{li}sp", bufs=3, space="PSUM") as pps,
    ):
        # ---- augmented X ----
        nc.vector.tensor_scalar(xl[0:C, :], xr[0:C, :], 2.0, None, ALU.mult)
        nc.vector.memset(xa1, 1.0)
        xsq = pc.tile([C, N], FP, tag="xsq")
        nc.scalar.activation(xsq, xr[0:C, :], AF.Square)
        ones_c = pc.tile([C, 1], FP, tag="ones_c")
        nc.vector.memset(ones_c, 1.0)
        for j in range(4):
            cs = slice(j * 512, (j + 1) * 512)
            sq_ps = pps.tile([1, 512], FP, tag="sps")
            nc.tensor.matmul(sq_ps, ones_c[0:C, :], xsq[0:C, cs],
                             start=True, stop=True)
            nc.scalar.activation(xan[:, cs], sq_ps, AF.Identity, scale=-1.0)
        if C < 128:
            # place aug rows at partition C (DMA: no base-partition limits)
            nc.sync.dma_start(out=xl[C:C + 1, :], in_=xa1)
            nc.sync.dma_start(out=xr[C:C + 1, :], in_=xan)

        # ---- weights ----
        w1t = pc.tile([C, O], FP, tag="w1t")
        w2t = pc.tile([C, O], FP, tag="w2t")
        tb = pc.tile([128, O], FP, tag="tb")
        nc.sync.dma_start(out=w1t, in_=w1[:, :])
        nc.sync.dma_start(out=w2t, in_=w2[:, :])
        nc.sync.dma_start(out=tb, in_=tbias[:, :])

        as_all = pc.tile([128, NT * O], FP, tag="as_all")
        bs_all = pc.tile([128, NT * O], FP, tag="bs_all")
        idx_all = pc.tile([128, NT * 24], mybir.dt.uint32, tag="idx_all")
        as_view = as_dram[:, :].rearrange("n o -> (n o)")[0:N * O]            .rearrange("(n o) -> n o", o=O)

        # ======== pass A1: As/Bs for all tiles (As -> DRAM + SBUF) ========
        for i in range(NT):
            nl = slice(i * 128, (i + 1) * 128)
            asps = pps.tile([128, O], FP, tag="sps")
            nc.tensor.matmul(asps, xr[0:C, nl], w1t[0:C, :], start=True, stop=True)
            as_sb = as_all[:, i * O:(i + 1) * O]
            nc.scalar.activation(as_sb, asps, AF.Identity)
            nc.sync.dma_start(out=as_view[nl, :], in_=as_sb)
            bsps = pps.tile([128, O], FP, tag="sps")
            nc.tensor.matmul(bsps, xr[0:C, nl], w2t[0:C, :], start=True, stop=True)
            nc.vector.tensor_tensor(bs_all[:, i * O:(i + 1) * O], bsps, tb, ALU.add)

        # ======== pass A2: u + topk per tile (Vector), feeding pass B ======
        for i in range(NT):
            nl = slice(i * 128, (i + 1) * 128)
            ups = ppu.tile([128, N], FP, tag="ups")
            for j in range(4):
                cs = slice(j * 512, (j + 1) * 512)
                if Ka <= 128:
                    nc.tensor.matmul(ups[:, cs], xl[0:Ka, nl], xr[0:Ka, cs],
                                     start=True, stop=True)
                else:
                    nc.tensor.matmul(ups[:, cs], xl[0:C, nl], xr[0:C, cs],
                                     start=True, stop=False)
                    nc.tensor.matmul(ups[:, cs], xa1[:, nl], xan[:, cs],
                                     start=False, stop=True)
            u = pud.tile([128, N], FP, tag="u")
            nc.scalar.activation(u, ups, AF.Identity)
            if "u0" in dbgt and li == 0 and i == 0:
                nc.sync.dma_start(out=dbgt["u0"][:, :], in_=u)

            vals = pool.tile([128, 24], FP, tag="vals")
            idxu = idx_all[:, i * 24:(i + 1) * 24]
            u2 = pu.tile([128, N], FP, tag="u2")
            u3 = pu.tile([128, N], FP, tag="u3")
            nc.vector.max(out=vals[:, 0:8], in_=u)
            nc.vector.max_index(out=idxu[:, 0:8], in_max=vals[:, 0:8], in_values=u)
            nc.vector.match_replace(out=u2, in_to_replace=vals[:, 0:8],
                                    in_values=u, imm_value=-1e30)
            nc.vector.max(out=vals[:, 8:16], in_=u2)
            nc.vector.max_index(out=idxu[:, 8:16], in_max=vals[:, 8:16],
                                in_values=u2)
            nc.vector.match_replace(out=u3, in_to_replace=vals[:, 8:16],
                                    in_values=u2, imm_value=-1e30)
            nc.vector.max(out=vals[:, 16:24], in_=u3)
            nc.vector.max_index(out=idxu[:, 16:24], in_max=vals[:, 16:24],
                                in_values=u3)
            if "idx0" in dbgt and li == 0 and i == 0:
                fi = pool.tile([128, 24], FP, tag="fi")
                nc.vector.tensor_copy(fi, idxu)
                nc.sync.dma_start(out=dbgt["idx0"][:, :], in_=fi)

        # ======== pass B: gather k=1..19 (k=0 is the node itself) =========
        for i in range(NT):
            nl = slice(i * 128, (i + 1) * 128)
            gath = pg.tile([128, K - 1, O], FP, tag="gath")
            for kk in range(1, K):
                nc.gpsimd.indirect_dma_start(
                    out=gath[:, kk - 1, :], out_offset=None,
                    in_=as_view[0:N, 0:O],
                    in_offset=bass.IndirectOffsetOnAxis(
                        ap=idx_all[:, i * 24 + kk:i * 24 + kk + 1], axis=0))
            m = pool.tile([128, O], FP, tag="m")
            nc.vector.tensor_reduce(m, gath.rearrange("p k o -> p o k"),
                                    mybir.AxisListType.X, ALU.max)
            nc.vector.tensor_tensor(m, m, as_all[:, i * O:(i + 1) * O], ALU.max)
            nc.vector.tensor_tensor(m, m, bs_all[:, i * O:(i + 1) * O], ALU.add)
            hl = pool.tile([128, O], FP, tag="hl")
            nc.vector.scalar_tensor_tensor(hl, m, NEG, m, ALU.mult, ALU.max)
            for ob in range((O + 127) // 128):
                osl = slice(ob * 128, min((ob + 1) * 128, O))
                ow = osl.stop - osl.start
                tps = pps.tile([128, 128], FP, tag="sps")
                nc.tensor.transpose(tps[0:ow, :], hl[:, osl], ident)
                if O > 128:
                    dst = h_out[0:ow, ob * N + i * 128:ob * N + (i + 1) * 128]
                else:
                    dst = h_out[osl, nl]
                nc.scalar.activation(dst, tps[0:ow, :], AF.Identity)

        if li < 3:
            nc.vector.tensor_copy(xr[0:O, :], h_out[0:O, :])
        if f"h{li}" in dbgt:
            if O > 128:
                nc.sync.dma_start(
                    out=dbgt[f"h{li}"][:, :].rearrange("(g p) n -> p g n", p=128),
                    in_=h_out.rearrange("p (g n) -> p g n", g=2))
            else:
                nc.sync.dma_start(out=dbgt[f"h{li}"][:, :], in_=h_out)


def _tail(nc, tc, hts, wc, t4, wl1, t5, wl2, t6, wl3, bl3, out, dbgt):
    with (
        tc.tile_pool(name="T", bufs=2) as pool,
        tc.tile_pool(name="Tc", bufs=1) as pc,
        tc.tile_pool(name="Tp", bufs=4, space="PSUM") as pps,
    ):
        # 5 K-pieces (h0, h1, h2, h3a, h3b), each loaded at base partition 0
        wct = pc.tile([128, 5 * 1024], FP, tag="wct")
        pieces = [(0, 64), (64, 128), (128, 256), (256, 384), (384, 512)]
        for pi, (r0, r1_) in enumerate(pieces):
            nc.sync.dma_start(out=wct[0:r1_ - r0, pi * 1024:(pi + 1) * 1024],
                              in_=wc[r0:r1_, :])
        t4t = pc.tile([128, 8], FP, tag="t4t")
        nc.sync.dma_start(out=t4t, in_=t4[:, :])
        # z rows: h0(64) h1(64) h2(128) h3(256): K-chunks of 128:
        # chunk0 = [h0; h1], chunk1 = h2, chunk2 = h3[0:128], chunk3 = h3[128:256]
        zmax = pc.tile([128, 8], FP, tag="zmax")
        zsum = pc.tile([128, 8], FP, tag="zsum")
        for mg in range(8):
            ml = slice(mg * 128, (mg + 1) * 128)
            z4g = pool.tile([128, N], FP, tag="z4g")
            for j in range(4):
                cs = slice(j * 512, (j + 1) * 512)
                zps = pps.tile([128, 512], FP, tag="zps")

                def wv(pi, kk):
                    return wct[0:kk, pi * 1024 + mg * 128:pi * 1024 + (mg + 1) * 128]

                nc.tensor.matmul(zps, wv(0, 64), hts[0][:, cs],
                                 start=True, stop=False)
                nc.tensor.matmul(zps, wv(1, 64), hts[1][:, cs],
                                 start=False, stop=False)
                nc.tensor.matmul(zps, wv(2, 128), hts[2][:, cs],
                                 start=False, stop=False)
                nc.tensor.matmul(zps, wv(3, 128), hts[3][:, cs],
                                 start=False, stop=False)
                nc.tensor.matmul(zps, wv(4, 128),
                                 hts[3][:, N + cs.start:N + cs.stop],
                                 start=False, stop=True)
                nc.scalar.activation(z4g[:, cs], zps, AF.Identity,
                                     bias=t4t[:, mg:mg + 1])
                nc.vector.scalar_tensor_tensor(z4g[:, cs], z4g[:, cs], NEG,
                                               z4g[:, cs], ALU.mult, ALU.max)
            if "z4" in dbgt and mg == 0:
                nc.sync.dma_start(out=dbgt["z4"][:, :], in_=z4g)
            nc.vector.tensor_reduce(zmax[:, mg:mg + 1], z4g,
                                    mybir.AxisListType.X, ALU.max)
            nc.vector.tensor_reduce(zsum[:, mg:mg + 1], z4g,
                                    mybir.AxisListType.X, ALU.add)
        if "zpool" in dbgt:
            zz = pool.tile([128, 16], FP, tag="zz")
            nc.vector.tensor_copy(zz[:, 0:8], zmax)
            nc.vector.tensor_copy(zz[:, 8:16], zsum)
            nc.sync.dma_start(out=dbgt["zpool"][:, :], in_=zz)

        # ---- FC tail (transposed matvecs, out stays [1, M]) ----
        def fc(z_cols, w_T, M, bias_row, lrelu):
            # z_cols: [128, nk] tile (K-chunks as columns); w_T: [128*nk, M] dram
            nk = z_cols.shape[1]
            ps = pps.tile([1, 512], FP, tag="fps")
            wt = pc.tile([128, nk * M], FP, tag=f"w{M}")
            nc.sync.dma_start(out=wt.rearrange("p (a o) -> p a o", o=M),
                              in_=w_T[:, :].rearrange("(a p) o -> p a o", p=128))
            for kc in range(nk):
                nc.tensor.matmul(ps[0:1, 0:M], z_cols[:, kc:kc + 1],
                                 wt[:, kc * M:(kc + 1) * M],
                                 start=(kc == 0), stop=(kc == nk - 1))
            row = pool.tile([1, M], FP, tag=f"row{M}")
            br = pool.tile([1, M], FP, tag=f"br{M}")
            nc.sync.dma_start(out=br, in_=bias_row[:, :])
            nc.scalar.activation(row, ps[0:1, 0:M], AF.Identity)
            nc.vector.tensor_tensor(row, row, br, ALU.add)
            if lrelu:
                nc.vector.scalar_tensor_tensor(row, row, NEG, row,
                                               ALU.mult, ALU.max)
            return row

        fcd = tc.tile_pool(name="fcd", bufs=2, space="DRAM")
        with fcd as fdp:
            z16 = pc.tile([128, 16], FP, tag="z16")
            nc.vector.tensor_copy(z16[:, 0:8], zmax)
            nc.vector.tensor_copy(z16[:, 8:16], zsum)
            r1 = fc(z16, wl1, 512, t5, True)                  # [1, 512]
            b1 = fdp.tile([512], FP, tag="fcb")
            nc.sync.dma_start(out=b1[:], in_=r1)
            z1c = pc.tile([128, 4], FP, tag="z1c")
            nc.sync.dma_start(out=z1c, in_=b1.rearrange("(a p) -> p a", p=128))
            r2 = fc(z1c, wl2, 256, t6, True)                  # [1, 256]
            b2 = fdp.tile([512], FP, tag="fcb")
            nc.sync.dma_start(out=b2[0:256], in_=r2)
            z2c = pc.tile([128, 2], FP, tag="z2c")
            nc.sync.dma_start(out=z2c,
                              in_=b2[0:256].rearrange("(a p) -> p a", p=128))
            r3 = fc(z2c, wl3, 40, bl3, False)                 # [1, 40]
            nc.sync.dma_start(out=out[:, :], in_=r3)


# ---------------------------------------------------------------------------
# Harness entry point: kernel(**inputs) -> np.ndarray [8, 40]
# ---------------------------------------------------------------------------
_NC_CACHE = {}


def _get_nc():
    if "nc" not in _NC_CACHE:
        _NC_CACHE["nc"] = build()
    return _NC_CACHE["nc"]


def kernel(**inputs):
    from concourse.bass_utils import run_bass_kernel_spmd

    nc = _get_nc()
    in_maps = [host_prep(inputs, core) for core in range(8)]
    res = run_bass_kernel_spmd(nc, in_maps, core_ids=list(range(8)))
    out = np.stack([r["out"][0] for r in res.results]).astype(np.float32)
    return out



# revision 5
# speedup vs baseline: 1.4309x; 1.3842x over previous
"""DGCNN Bass kernel for TRN2 — one sample per NeuronCore.

Math (per graph-conv layer, BN folded on host):
  u[n,m] = 2<x_n,x_m> - |x_m|^2   (row-wise top-20 == reference kNN)
  As = X^T (s*W1)^T [N,O];  Bs = X^T (s*(W2-W1))^T + t [N,O]
  h[n,o] = LReLU(max_{j in knn(n)} As[j,o] + Bs[n,o]);  X_next = h^T
Tail: conv1d(512->1024)+BN+LReLU, global max/avg pool, FC 2048->512->256->40.
"""
import numpy as np

import concourse.bass as bass
import concourse.bacc as bacc
import concourse.mybir as mybir
from concourse.tile import TileContext
from concourse.masks import make_identity

FP = mybir.dt.float32
AF = mybir.ActivationFunctionType
ALU = mybir.AluOpType
N = 2048
K = 20
NEG = 0.2
NT = N // 128  # 16 row tiles

LAYERS = [(3, 64), (64, 64), (64, 128), (128, 256)]  # (C_in, O)


def host_prep(inputs, core):
    """Per-core parameter dict from the full input dict (numpy arrays)."""
    d = {k: np.asarray(v, np.float32) for k, v in inputs.items()}
    p = {"x": np.ascontiguousarray(d["x"][core])}  # [3, N]
    for li, (c, o) in enumerate(LAYERS):
        w = d[f"w_gc{li}"]                      # [O, 2C]
        s = d[f"s{li}"]
        t = d[f"t{li}"]
        w1 = w[:, :c] * s[:, None]              # [O, C]
        w2 = (w[:, c:] - w[:, :c]) * s[:, None]
        p[f"w1_{li}"] = np.ascontiguousarray(w1.T)     # [C, O]
        p[f"w2_{li}"] = np.ascontiguousarray(w2.T)
        p[f"t_{li}"] = np.ascontiguousarray(
            np.broadcast_to(t[None, :], (128, o)))     # replicated bias
    p["w_c1d_T"] = np.ascontiguousarray((d["w_c1d"] * d["s4"][:, None]).T)  # [512,1024]
    p["t4"] = np.ascontiguousarray(d["t4"].reshape(8, 128).T)  # [128, 8] per-mg cols
    wl1 = d["w_l1"] * d["s5"][:, None]          # [512, 2048]
    wl1 = wl1.copy()
    wl1[:, 1024:] /= float(N)                   # fold avg-pool divisor
    p["w_l1_T"] = np.ascontiguousarray(wl1.T)   # [2048, 512]
    p["t5"] = np.ascontiguousarray(d["t5"][None, :])   # [1, 512]
    wl2 = d["w_l2"] * d["s6"][:, None]
    p["w_l2_T"] = np.ascontiguousarray(wl2.T)   # [512, 256]
    p["t6"] = np.ascontiguousarray(
        (d["s6"] * d["b_l2"] + d["t6"])[None, :])      # [1, 256]
    p["w_l3_T"] = np.ascontiguousarray(d["w_l3"].T)    # [256, 40]
    p["b_l3"] = np.ascontiguousarray(d["b_l3"][None, :])
    return p


def build(dbg=()):
    nc = bacc.Bacc(None, target_bir_lowering=False, num_swdge_queues=4)
    x_in = nc.declare_dram_parameter("x", [3, N], FP, isOutput=False)
    W1, W2, TB = {}, {}, {}
    for li, (c, o) in enumerate(LAYERS):
        W1[li] = nc.declare_dram_parameter(f"w1_{li}", [c, o], FP, isOutput=False)
        W2[li] = nc.declare_dram_parameter(f"w2_{li}", [c, o], FP, isOutput=False)
        TB[li] = nc.declare_dram_parameter(f"t_{li}", [128, o], FP, isOutput=False)
    wc = nc.declare_dram_parameter("w_c1d_T", [512, 1024], FP, isOutput=False)
    t4 = nc.declare_dram_parameter("t4", [128, 8], FP, isOutput=False)
    wl1 = nc.declare_dram_parameter("w_l1_T", [2048, 512], FP, isOutput=False)
    t5 = nc.declare_dram_parameter("t5", [1, 512], FP, isOutput=False)
    wl2 = nc.declare_dram_parameter("w_l2_T", [512, 256], FP, isOutput=False)
    t6 = nc.declare_dram_parameter("t6", [1, 256], FP, isOutput=False)
    wl3 = nc.declare_dram_parameter("w_l3_T", [256, 40], FP, isOutput=False)
    bl3 = nc.declare_dram_parameter("b_l3", [1, 40], FP, isOutput=False)
    out = nc.declare_dram_parameter("out", [1, 40], FP, isOutput=True)

    dbgt = {}
    for name in dbg:
        shp = {"u0": [128, N], "idx0": [128, 24],
               "h0": [64, N], "h1": [64, N], "h2": [128, N], "h3": [256, N],
               "z4": [128, N], "zpool": [128, 16]}[name]
        dbgt[name] = nc.declare_dram_parameter("dbg_" + name, shp, FP, isOutput=True)

    with TileContext(nc) as tc:
        with (
            tc.tile_pool(name="const", bufs=1) as cp,
            tc.tile_pool(name="dram", bufs=2, space="DRAM") as dp,
            tc.tile_pool(name="x", bufs=1) as xp,
        ):
            ident = cp.tile([128, 128], FP)
            make_identity(nc, ident)
            xl = xp.tile([128, N], FP)   # [2X; ones(row C) for C<128]
            xr = xp.tile([128, N], FP)   # [X; -sq(row C) for C<128]
            xa1 = xp.tile([1, N], FP)    # ones row (L4)
            xan = xp.tile([1, N], FP)    # -sq row (L4)
            hts = []
            for li in range(4):
                o_l = LAYERS[li][1]
                h_t = xp.tile([min(o_l, 128), (o_l // 128 or 1) * N], FP,
                              tag=f"h{li}")
                hts.append(h_t)
            nc.sync.dma_start(out=xr[0:3, :], in_=x_in[:, :])

            for li, (C, O) in enumerate(LAYERS):
                _layer(nc, tc, dp, li, C, O, xl, xr, xa1, xan, ident,
                       W1[li], W2[li], TB[li], hts[li], dbgt)

            _tail(nc, tc, hts, wc, t4, wl1, t5, wl2, t6, wl3, bl3, out, dbgt)
    nc.finalize()
    return nc


def _layer(nc, tc, dp, li, C, O, xl, xr, xa1, xan, ident, w1, w2, tbias,
           h_out, dbgt):
    Ka = C + 1
    as_dram = dp.tile([N, 256], FP, tag="as_dram")
    with (
        tc.tile_pool(name=f"L{li}", bufs=3) as pool,
        tc.tile_pool(name=f"L{li}u", bufs=1) as pu,
        tc.tile_pool(name=f"L{li}ud", bufs=2) as pud,
        tc.tile_pool(name=f"L{li}c", bufs=1) as pc,
        tc.tile_pool(name=f"L{li}g", bufs=2 if li >= 2 else 3) as pg,
        tc.tile_pool(name=f"L{li}up", bufs=1, space="PSUM") as ppu,
        tc.tile_pool(name=f"L{li}sp", bufs=3, space="PSUM") as pps,
    ):
        # ---- augmented X ----
        nc.vector.tensor_scalar(xl[0:C, :], xr[0:C, :], 2.0, None, ALU.mult)
        nc.vector.memset(xa1, 1.0)
        xsq = pc.tile([C, N], FP, tag="xsq")
        nc.scalar.activation(xsq, xr[0:C, :], AF.Square)
        ones_c = pc.tile([C, 1], FP, tag="ones_c")
        nc.vector.memset(ones_c, 1.0)
        for j in range(4):
            cs = slice(j * 512, (j + 1) * 512)
            sq_ps = pps.tile([1, 512], FP, tag="sps")
            nc.tensor.matmul(sq_ps, ones_c[0:C, :], xsq[0:C, cs],
                             start=True, stop=True)
            nc.scalar.activation(xan[:, cs], sq_ps, AF.Identity, scale=-1.0)
        if C < 128:
            # place aug rows at partition C (DMA: no base-partition limits)
            nc.sync.dma_start(out=xl[C:C + 1, :], in_=xa1)
            nc.sync.dma_start(out=xr[C:C + 1, :], in_=xan)

        # ---- weights ----
        w1t = pc.tile([C, O], FP, tag="w1t")
        w2t = pc.tile([C, O], FP, tag="w2t")
        tb = pc.tile([128, O], FP, tag="tb")
        nc.sync.dma_start(out=w1t, in_=w1[:, :])
        nc.sync.dma_start(out=w2t, in_=w2[:, :])
        nc.sync.dma_start(out=tb, in_=tbias[:, :])

        as_all = pc.tile([128, NT * O], FP, tag="as_all")
        bs_all = pc.tile([128, NT * O], FP, tag="bs_all")
        idx_all = pc.tile([128, NT * 24], mybir.dt.uint32, tag="idx_all")
        as_view = as_dram[:, :].rearrange("n o -> (n o)")[0:N * O]            .rearrange("(n o) -> n o", o=O)

        # ======== pass A1: As/Bs for all tiles (As -> DRAM + SBUF) ========
        for i in range(NT):
            nl = slice(i * 128, (i + 1) * 128)
            asps = pps.tile([128, O], FP, tag="sps")
            nc.tensor.matmul(asps, xr[0:C, nl], w1t[0:C, :], start=True, stop=True)
            as_sb = as_all[:, i * O:(i + 1) * O]
            nc.scalar.activation(as_sb, asps, AF.Identity)
            nc.sync.dma_start(out=as_view[nl, :], in_=as_sb)
            bsps = pps.tile([128, O], FP, tag="sps")
            nc.tensor.matmul(bsps, xr[0:C, nl], w2t[0:C, :], start=True, stop=True)
            nc.vector.tensor_tensor(bs_all[:, i * O:(i + 1) * O], bsps, tb, ALU.add)

        # ==== pass A2+B fused, software-pipelined with LAG ====
        # topk_i (Vector) interleaves with reduce_{i-LAG} so the gather
        # pool recycles while GpSimd streams INDIRECT1Ds continuously.
        LAG = 2
        gaths = [None] * NT

        def emit_topk_and_gather(i):
            nl = slice(i * 128, (i + 1) * 128)
            ups = ppu.tile([128, N], FP, tag="ups")
            for j in range(4):
                cs = slice(j * 512, (j + 1) * 512)
                if Ka <= 128:
                    nc.tensor.matmul(ups[:, cs], xl[0:Ka, nl], xr[0:Ka, cs],
                                     start=True, stop=True)
                else:
                    nc.tensor.matmul(ups[:, cs], xl[0:C, nl], xr[0:C, cs],
                                     start=True, stop=False)
                    nc.tensor.matmul(ups[:, cs], xa1[:, nl], xan[:, cs],
                                     start=False, stop=True)
            u = pud.tile([128, N], FP, tag="u")
            nc.scalar.activation(u, ups, AF.Identity)
            if "u0" in dbgt and li == 0 and i == 0:
                nc.sync.dma_start(out=dbgt["u0"][:, :], in_=u)

            vals = pool.tile([128, 24], FP, tag="vals")
            idxu = idx_all[:, i * 24:(i + 1) * 24]
            u2 = pu.tile([128, N], FP, tag="u2")
            u3 = pu.tile([128, N], FP, tag="u3")
            nc.vector.max(out=vals[:, 0:8], in_=u)
            nc.vector.max_index(out=idxu[:, 0:8], in_max=vals[:, 0:8], in_values=u)
            nc.vector.match_replace(out=u2, in_to_replace=vals[:, 0:8],
                                    in_values=u, imm_value=-1e30)
            nc.vector.max(out=vals[:, 8:16], in_=u2)
            nc.vector.max_index(out=idxu[:, 8:16], in_max=vals[:, 8:16],
                                in_values=u2)
            nc.vector.match_replace(out=u3, in_to_replace=vals[:, 8:16],
                                    in_values=u2, imm_value=-1e30)
            nc.vector.max(out=vals[:, 16:24], in_=u3)
            nc.vector.max_index(out=idxu[:, 16:24], in_max=vals[:, 16:24],
                                in_values=u3)
            if "idx0" in dbgt and li == 0 and i == 0:
                fi = pool.tile([128, 24], FP, tag="fi")
                nc.vector.tensor_copy(fi, idxu)
                nc.sync.dma_start(out=dbgt["idx0"][:, :], in_=fi)

            # gather k=1..19 (k=0 is the node itself, already in as_all)
            gath = pg.tile([128, K - 1, O], FP, tag="gath")
            gaths[i] = gath
            for kk in range(1, K):
                nc.gpsimd.indirect_dma_start(
                    out=gath[:, kk - 1, :], out_offset=None,
                    in_=as_view[0:N, 0:O],
                    in_offset=bass.IndirectOffsetOnAxis(
                        ap=idx_all[:, i * 24 + kk:i * 24 + kk + 1], axis=0))

        def emit_reduce(i):
            nl = slice(i * 128, (i + 1) * 128)
            gath = gaths[i]
            m = pool.tile([128, O], FP, tag="m")
            nc.vector.tensor_reduce(m, gath.rearrange("p k o -> p o k"),
                                    mybir.AxisListType.X, ALU.max)
            nc.vector.tensor_tensor(m, m, as_all[:, i * O:(i + 1) * O], ALU.max)
            nc.vector.tensor_tensor(m, m, bs_all[:, i * O:(i + 1) * O], ALU.add)
            hl = pool.tile([128, O], FP, tag="hl")
            nc.vector.scalar_tensor_tensor(hl, m, NEG, m, ALU.mult, ALU.max)
            for ob in range((O + 127) // 128):
                osl = slice(ob * 128, min((ob + 1) * 128, O))
                ow = osl.stop - osl.start
                tps = pps.tile([128, 128], FP, tag="sps")
                nc.tensor.transpose(tps[0:ow, :], hl[:, osl], ident)
                if O > 128:
                    dst = h_out[0:ow, ob * N + i * 128:ob * N + (i + 1) * 128]
                else:
                    dst = h_out[osl, nl]
                nc.scalar.activation(dst, tps[0:ow, :], AF.Identity)

        for i in range(NT + LAG):
            if i < NT:
                emit_topk_and_gather(i)
            if i - LAG >= 0:
                emit_reduce(i - LAG)

        if li < 3:
            nc.vector.tensor_copy(xr[0:O, :], h_out[0:O, :])
        if f"h{li}" in dbgt:
            if O > 128:
                nc.sync.dma_start(
                    out=dbgt[f"h{li}"][:, :].rearrange("(g p) n -> p g n", p=128),
                    in_=h_out.rearrange("p (g n) -> p g n", g=2))
            else:
                nc.sync.dma_start(out=dbgt[f"h{li}"][:, :], in_=h_out)


def _tail(nc, tc, hts, wc, t4, wl1, t5, wl2, t6, wl3, bl3, out, dbgt):
    with (
        tc.tile_pool(name="T", bufs=2) as pool,
        tc.tile_pool(name="Tc", bufs=1) as pc,
        tc.tile_pool(name="Tp", bufs=4, space="PSUM") as pps,
    ):
        # 5 K-pieces (h0, h1, h2, h3a, h3b), each loaded at base partition 0
        wct = pc.tile([128, 5 * 1024], FP, tag="wct")
        pieces = [(0, 64), (64, 128), (128, 256), (256, 384), (384, 512)]
        for pi, (r0, r1_) in enumerate(pieces):
            nc.sync.dma_start(out=wct[0:r1_ - r0, pi * 1024:(pi + 1) * 1024],
                              in_=wc[r0:r1_, :])
        t4t = pc.tile([128, 8], FP, tag="t4t")
        nc.sync.dma_start(out=t4t, in_=t4[:, :])
        # z rows: h0(64) h1(64) h2(128) h3(256): K-chunks of 128:
        # chunk0 = [h0; h1], chunk1 = h2, chunk2 = h3[0:128], chunk3 = h3[128:256]
        zmax = pc.tile([128, 8], FP, tag="zmax")
        zsum = pc.tile([128, 8], FP, tag="zsum")
        for mg in range(8):
            ml = slice(mg * 128, (mg + 1) * 128)
            z4g = pool.tile([128, N], FP, tag="z4g")
            for j in range(4):
                cs = slice(j * 512, (j + 1) * 512)
                zps = pps.tile([128, 512], FP, tag="zps")

                def wv(pi, kk):
                    return wct[0:kk, pi * 1024 + mg * 128:pi * 1024 + (mg + 1) * 128]

                nc.tensor.matmul(zps, wv(0, 64), hts[0][:, cs],
                                 start=True, stop=False)
                nc.tensor.matmul(zps, wv(1, 64), hts[1][:, cs],
                                 start=False, stop=False)
                nc.tensor.matmul(zps, wv(2, 128), hts[2][:, cs],
                                 start=False, stop=False)
                nc.tensor.matmul(zps, wv(3, 128), hts[3][:, cs],
                                 start=False, stop=False)
                nc.tensor.matmul(zps, wv(4, 128),
                                 hts[3][:, N + cs.start:N + cs.stop],
                                 start=False, stop=True)
                nc.scalar.activation(z4g[:, cs], zps, AF.Identity,
                                     bias=t4t[:, mg:mg + 1])
                nc.vector.scalar_tensor_tensor(z4g[:, cs], z4g[:, cs], NEG,
                                               z4g[:, cs], ALU.mult, ALU.max)
            if "z4" in dbgt and mg == 0:
                nc.sync.dma_start(out=dbgt["z4"][:, :], in_=z4g)
            nc.vector.tensor_reduce(zmax[:, mg:mg + 1], z4g,
                                    mybir.AxisListType.X, ALU.max)
            nc.vector.tensor_reduce(zsum[:, mg:mg + 1], z4g,
                                    mybir.AxisListType.X, ALU.add)
        if "zpool" in dbgt:
            zz = pool.tile([128, 16], FP, tag="zz")
            nc.vector.tensor_copy(zz[:, 0:8], zmax)
            nc.vector.tensor_copy(zz[:, 8:16], zsum)
            nc.sync.dma_start(out=dbgt["zpool"][:, :], in_=zz)

        # ---- FC tail (transposed matvecs, out stays [1, M]) ----
        def fc(z_cols, w_T, M, bias_row, lrelu):
            # z_cols: [128, nk] tile (K-chunks as columns); w_T: [128*nk, M] dram
            nk = z_cols.shape[1]
            ps = pps.tile([1, 512], FP, tag="fps")
            wt = pc.tile([128, nk * M], FP, tag=f"w{M}")
            nc.sync.dma_start(out=wt.rearrange("p (a o) -> p a o", o=M),
                              in_=w_T[:, :].rearrange("(a p) o -> p a o", p=128))
            for kc in range(nk):
                nc.tensor.matmul(ps[0:1, 0:M], z_cols[:, kc:kc + 1],
                                 wt[:, kc * M:(kc + 1) * M],
                                 start=(kc == 0), stop=(kc == nk - 1))
            row = pool.tile([1, M], FP, tag=f"row{M}")
            br = pool.tile([1, M], FP, tag=f"br{M}")
            nc.sync.dma_start(out=br, in_=bias_row[:, :])
            nc.scalar.activation(row, ps[0:1, 0:M], AF.Identity)
            nc.vector.tensor_tensor(row, row, br, ALU.add)
            if lrelu:
                nc.vector.scalar_tensor_tensor(row, row, NEG, row,
                                               ALU.mult, ALU.max)
            return row

        fcd = tc.tile_pool(name="fcd", bufs=2, space="DRAM")
        with fcd as fdp:
            z16 = pc.tile([128, 16], FP, tag="z16")
            nc.vector.tensor_copy(z16[:, 0:8], zmax)
            nc.vector.tensor_copy(z16[:, 8:16], zsum)
            r1 = fc(z16, wl1, 512, t5, True)                  # [1, 512]
            b1 = fdp.tile([512], FP, tag="fcb")
            nc.sync.dma_start(out=b1[:], in_=r1)
            z1c = pc.tile([128, 4], FP, tag="z1c")
            nc.sync.dma_start(out=z1c, in_=b1.rearrange("(a p) -> p a", p=128))
            r2 = fc(z1c, wl2, 256, t6, True)                  # [1, 256]
            b2 = fdp.tile([512], FP, tag="fcb")
            nc.sync.dma_start(out=b2[0:256], in_=r2)
            z2c = pc.tile([128, 2], FP, tag="z2c")
            nc.sync.dma_start(out=z2c,
                              in_=b2[0:256].rearrange("(a p) -> p a", p=128))
            r3 = fc(z2c, wl3, 40, bl3, False)                 # [1, 40]
            nc.sync.dma_start(out=out[:, :], in_=r3)


# ---------------------------------------------------------------------------
# Harness entry point: kernel(**inputs) -> np.ndarray [8, 40]
# ---------------------------------------------------------------------------
_NC_CACHE = {}


def _get_nc():
    if "nc" not in _NC_CACHE:
        _NC_CACHE["nc"] = build()
    return _NC_CACHE["nc"]


def kernel(**inputs):
    from concourse.bass_utils import run_bass_kernel_spmd

    nc = _get_nc()
    in_maps = [host_prep(inputs, core) for core in range(8)]
    res = run_bass_kernel_spmd(nc, in_maps, core_ids=list(range(8)))
    out = np.stack([r["out"][0] for r in res.results]).astype(np.float32)
    return out



# revision 15
# speedup vs baseline: 1.4745x; 1.0305x over previous
"""DGCNN Bass kernel for TRN2 — one sample per NeuronCore.

Math (per graph-conv layer, BN folded on host):
  u[n,m] = 2<x_n,x_m> - |x_m|^2   (row-wise top-20 == reference kNN)
  As = X^T (s*W1)^T [N,O];  Bs = X^T (s*(W2-W1))^T + t [N,O]
  h[n,o] = LReLU(max_{j in knn(n)} As[j,o] + Bs[n,o]);  X_next = h^T
Tail: conv1d(512->1024)+BN+LReLU, global max/avg pool, FC 2048->512->256->40.
"""
import numpy as np

import concourse.bass as bass
import concourse.bacc as bacc
import concourse.mybir as mybir
from concourse.tile import TileContext
from concourse.masks import make_identity

FP = mybir.dt.float32
AF = mybir.ActivationFunctionType
ALU = mybir.AluOpType
N = 2048
K = 20
NEG = 0.2
NT = N // 128  # 16 row tiles

LAYERS = [(3, 64), (64, 64), (64, 128), (128, 256)]  # (C_in, O)


def host_prep(inputs, core):
    """Per-core parameter dict from the full input dict (numpy arrays)."""
    d = {k: np.asarray(v, np.float32) for k, v in inputs.items()}
    p = {"x": np.ascontiguousarray(d["x"][core])}  # [3, N]
    for li, (c, o) in enumerate(LAYERS):
        w = d[f"w_gc{li}"]                      # [O, 2C]
        s = d[f"s{li}"]
        t = d[f"t{li}"]
        w1 = w[:, :c] * s[:, None]              # [O, C]
        w2 = (w[:, c:] - w[:, :c]) * s[:, None]
        p[f"w1_{li}"] = np.ascontiguousarray(w1.T)     # [C, O]
        p[f"w2_{li}"] = np.ascontiguousarray(w2.T)
        p[f"t_{li}"] = np.ascontiguousarray(
            np.broadcast_to(t[None, :], (128, o)))     # replicated bias
    p["w_c1d_T"] = np.ascontiguousarray((d["w_c1d"] * d["s4"][:, None]).T)  # [512,1024]
    p["t4"] = np.ascontiguousarray(d["t4"].reshape(8, 128).T)  # [128, 8] per-mg cols
    wl1 = d["w_l1"] * d["s5"][:, None]          # [512, 2048]
    wl1 = wl1.copy()
    wl1[:, 1024:] /= float(N)                   # fold avg-pool divisor
    p["w_l1_T"] = np.ascontiguousarray(wl1.T)   # [2048, 512]
    p["t5"] = np.ascontiguousarray(d["t5"][None, :])   # [1, 512]
    wl2 = d["w_l2"] * d["s6"][:, None]
    p["w_l2_T"] = np.ascontiguousarray(wl2.T)   # [512, 256]
    p["t6"] = np.ascontiguousarray(
        (d["s6"] * d["b_l2"] + d["t6"])[None, :])      # [1, 256]
    p["w_l3_T"] = np.ascontiguousarray(d["w_l3"].T)    # [256, 40]
    p["b_l3"] = np.ascontiguousarray(d["b_l3"][None, :])
    return p


def build(dbg=()):
    nc = bacc.Bacc(None, target_bir_lowering=False, num_swdge_queues=4)
    x_in = nc.declare_dram_parameter("x", [3, N], FP, isOutput=False)
    W1, W2, TB = {}, {}, {}
    for li, (c, o) in enumerate(LAYERS):
        W1[li] = nc.declare_dram_parameter(f"w1_{li}", [c, o], FP, isOutput=False)
        W2[li] = nc.declare_dram_parameter(f"w2_{li}", [c, o], FP, isOutput=False)
        TB[li] = nc.declare_dram_parameter(f"t_{li}", [128, o], FP, isOutput=False)
    wc = nc.declare_dram_parameter("w_c1d_T", [512, 1024], FP, isOutput=False)
    t4 = nc.declare_dram_parameter("t4", [128, 8], FP, isOutput=False)
    wl1 = nc.declare_dram_parameter("w_l1_T", [2048, 512], FP, isOutput=False)
    t5 = nc.declare_dram_parameter("t5", [1, 512], FP, isOutput=False)
    wl2 = nc.declare_dram_parameter("w_l2_T", [512, 256], FP, isOutput=False)
    t6 = nc.declare_dram_parameter("t6", [1, 256], FP, isOutput=False)
    wl3 = nc.declare_dram_parameter("w_l3_T", [256, 40], FP, isOutput=False)
    bl3 = nc.declare_dram_parameter("b_l3", [1, 40], FP, isOutput=False)
    out = nc.declare_dram_parameter("out", [1, 40], FP, isOutput=True)

    dbgt = {}
    for name in dbg:
        shp = {"u0": [128, N], "idx0": [128, 24],
               "h0": [64, N], "h1": [64, N], "h2": [128, N], "h3": [256, N],
               "z4": [128, N], "zpool": [128, 16]}[name]
        dbgt[name] = nc.declare_dram_parameter("dbg_" + name, shp, FP, isOutput=True)

    with TileContext(nc) as tc:
        with (
            tc.tile_pool(name="const", bufs=1) as cp,
            tc.tile_pool(name="dram", bufs=2, space="DRAM") as dp,
            tc.tile_pool(name="x", bufs=1) as xp,
        ):
            ident = cp.tile([128, 128], FP)
            make_identity(nc, ident)
            xl = xp.tile([128, N], FP)   # [2X; ones(row C) for C<128]
            xr = xp.tile([128, N], FP)   # [X; -sq(row C) for C<128]
            xa1 = xp.tile([1, N], FP)    # ones row (L4)
            xan = xp.tile([1, N], FP)    # -sq row (L4)
            hts = []
            for li in range(4):
                o_l = LAYERS[li][1]
                h_t = xp.tile([min(o_l, 128), (o_l // 128 or 1) * N], FP,
                              tag=f"h{li}")
                hts.append(h_t)
            nc.sync.dma_start(out=xr[0:3, :], in_=x_in[:, :])

            for li in range(3):
                C, O = LAYERS[li]
                _layer(nc, tc, dp, li, C, O, xl, xr, xa1, xan, ident,
                       W1[li], W2[li], TB[li], hts[li], dbgt)

            # L3 with the conv1d+pool tail overlapped into its gather window
            with (
                tc.tile_pool(name="Tc", bufs=1) as tpc,
                tc.tile_pool(name="Tw", bufs=2) as tpw,
                tc.tile_pool(name="Tp", bufs=2, space="PSUM") as tpp,
                tc.tile_pool(name="Td", bufs=2, space="DRAM") as tdp,
            ):
                tail = _TailOverlap(nc, tpc, tpw, tpp, hts, wc, t4)
                C, O = LAYERS[3]
                _layer(nc, tc, dp, 3, C, O, xl, xr, xa1, xan, ident,
                       W1[3], W2[3], TB[3], hts[3], dbgt,
                       post_reduce_cb=tail.chunk)
                tail.finish(tc, tdp, wl1, t5, wl2, t6, wl3, bl3, out, dbgt)
    nc.finalize()
    return nc


def _layer(nc, tc, dp, li, C, O, xl, xr, xa1, xan, ident, w1, w2, tbias,
           h_out, dbgt, post_reduce_cb=None):
    Ka = C + 1
    use_self = li < 3          # skip k=0 gather via SBUF As (SBUF-tight on L3)
    k0 = 1 if use_self else 0
    as_dram = dp.tile([N, 256], FP, tag="as_dram")
    with (
        tc.tile_pool(name=f"L{li}", bufs=3) as pool,
        tc.tile_pool(name=f"L{li}u", bufs=1) as pu,
        tc.tile_pool(name=f"L{li}ud", bufs=2 if li < 3 else 1) as pud,
        tc.tile_pool(name=f"L{li}c", bufs=1) as pc,
        tc.tile_pool(name=f"L{li}g", bufs=2 if li >= 2 else 3) as pg,
        tc.tile_pool(name=f"L{li}up", bufs=1, space="PSUM") as ppu,
        tc.tile_pool(name=f"L{li}sp", bufs=3 if li < 3 else 2,
                     space="PSUM") as pps,
    ):
        # ---- augmented X ----
        nc.vector.tensor_scalar(xl[0:C, :], xr[0:C, :], 2.0, None, ALU.mult)
        nc.vector.memset(xa1, 1.0)
        xsq = pc.tile([C, N], FP, tag="xsq")
        nc.scalar.activation(xsq, xr[0:C, :], AF.Square)
        ones_c = pc.tile([C, 1], FP, tag="ones_c")
        nc.vector.memset(ones_c, 1.0)
        for j in range(4):
            cs = slice(j * 512, (j + 1) * 512)
            sq_ps = pps.tile([1, 512], FP, tag="sps")
            nc.tensor.matmul(sq_ps, ones_c[0:C, :], xsq[0:C, cs],
                             start=True, stop=True)
            nc.scalar.activation(xan[:, cs], sq_ps, AF.Identity, scale=-1.0)
        if C < 128:
            # place aug rows at partition C (DMA: no base-partition limits)
            nc.sync.dma_start(out=xl[C:C + 1, :], in_=xa1)
            nc.sync.dma_start(out=xr[C:C + 1, :], in_=xan)

        # ---- weights ----
        w1t = pc.tile([C, O], FP, tag="w1t")
        w2t = pc.tile([C, O], FP, tag="w2t")
        tb = pc.tile([128, O], FP, tag="tb")
        nc.sync.dma_start(out=w1t, in_=w1[:, :])
        nc.sync.dma_start(out=w2t, in_=w2[:, :])
        nc.sync.dma_start(out=tb, in_=tbias[:, :])

        if use_self:
            as_all = pc.tile([128, NT * O], FP, tag="as_all")
        else:
            as_all = None
        bs_all = pc.tile([128, NT * O], FP, tag="bs_all")
        idx_all = pc.tile([128, NT * 24], mybir.dt.uint32, tag="idx_all")
        as_view = as_dram[:, :].rearrange("n o -> (n o)")[0:N * O]            .rearrange("(n o) -> n o", o=O)

        # ======== pass A1: As/Bs for all tiles (As -> DRAM + SBUF) ========
        for i in range(NT):
            nl = slice(i * 128, (i + 1) * 128)
            asps = pps.tile([128, O], FP, tag="sps")
            nc.tensor.matmul(asps, xr[0:C, nl], w1t[0:C, :], start=True, stop=True)
            if use_self:
                as_sb = as_all[:, i * O:(i + 1) * O]
            else:
                as_sb = pool.tile([128, O], FP, tag="as_sb")
            nc.scalar.activation(as_sb, asps, AF.Identity)
            nc.sync.dma_start(out=as_view[nl, :], in_=as_sb)
            bsps = pps.tile([128, O], FP, tag="sps")
            nc.tensor.matmul(bsps, xr[0:C, nl], w2t[0:C, :], start=True, stop=True)
            nc.vector.tensor_tensor(bs_all[:, i * O:(i + 1) * O], bsps, tb, ALU.add)

        # ==== pass A2+B fused, software-pipelined with LAG ====
        # topk_i (Vector) interleaves with reduce_{i-LAG} so the gather
        # pool recycles while GpSimd streams INDIRECT1Ds continuously.
        LAG = 2
        gaths = [None] * NT

        def emit_topk_and_gather(i):
            nl = slice(i * 128, (i + 1) * 128)
            ups = ppu.tile([128, N], FP, tag="ups")
            for j in range(4):
                cs = slice(j * 512, (j + 1) * 512)
                if Ka <= 128:
                    nc.tensor.matmul(ups[:, cs], xl[0:Ka, nl], xr[0:Ka, cs],
                                     start=True, stop=True)
                else:
                    nc.tensor.matmul(ups[:, cs], xl[0:C, nl], xr[0:C, cs],
                                     start=True, stop=False)
                    nc.tensor.matmul(ups[:, cs], xa1[:, nl], xan[:, cs],
                                     start=False, stop=True)
            u = pud.tile([128, N], FP, tag="u")
            nc.scalar.activation(u, ups, AF.Identity)
            if "u0" in dbgt and li == 0 and i == 0:
                nc.sync.dma_start(out=dbgt["u0"][:, :], in_=u)

            vals = pool.tile([128, 24], FP, tag="vals")
            idxu = idx_all[:, i * 24:(i + 1) * 24]
            u2 = pu.tile([128, N], FP, tag="u2")
            u3 = pu.tile([128, N], FP, tag="u3")
            nc.vector.max(out=vals[:, 0:8], in_=u)
            nc.vector.max_index(out=idxu[:, 0:8], in_max=vals[:, 0:8], in_values=u)
            nc.vector.match_replace(out=u2, in_to_replace=vals[:, 0:8],
                                    in_values=u, imm_value=-1e30)
            nc.vector.max(out=vals[:, 8:16], in_=u2)
            nc.vector.max_index(out=idxu[:, 8:16], in_max=vals[:, 8:16],
                                in_values=u2)
            nc.vector.match_replace(out=u3, in_to_replace=vals[:, 8:16],
                                    in_values=u2, imm_value=-1e30)
            nc.vector.max(out=vals[:, 16:24], in_=u3)
            nc.vector.max_index(out=idxu[:, 16:24], in_max=vals[:, 16:24],
                                in_values=u3)
            if "idx0" in dbgt and li == 0 and i == 0:
                fi = pool.tile([128, 24], FP, tag="fi")
                nc.vector.tensor_copy(fi, idxu)
                nc.sync.dma_start(out=dbgt["idx0"][:, :], in_=fi)

            # gather k=k0..19 (k=0 is the node itself, in as_all when kept)
            gath = pg.tile([128, K - k0, O], FP, tag="gath")
            gaths[i] = gath
            for kk in range(k0, K):
                nc.gpsimd.indirect_dma_start(
                    out=gath[:, kk - k0, :], out_offset=None,
                    in_=as_view[0:N, 0:O],
                    in_offset=bass.IndirectOffsetOnAxis(
                        ap=idx_all[:, i * 24 + kk:i * 24 + kk + 1], axis=0))

        def emit_reduce(i):
            nl = slice(i * 128, (i + 1) * 128)
            gath = gaths[i]
            m = pool.tile([128, O], FP, tag="m")
            nc.vector.tensor_reduce(m, gath.rearrange("p k o -> p o k"),
                                    mybir.AxisListType.X, ALU.max)
            if use_self:
                nc.vector.tensor_tensor(m, m, as_all[:, i * O:(i + 1) * O],
                                        ALU.max)
            nc.vector.tensor_tensor(m, m, bs_all[:, i * O:(i + 1) * O], ALU.add)
            hl = pool.tile([128, O], FP, tag="hl")
            nc.vector.scalar_tensor_tensor(hl, m, NEG, m, ALU.mult, ALU.max)
            for ob in range((O + 127) // 128):
                osl = slice(ob * 128, min((ob + 1) * 128, O))
                ow = osl.stop - osl.start
                tps = pps.tile([128, 128], FP, tag="sps")
                nc.tensor.transpose(tps[0:ow, :], hl[:, osl], ident)
                if O > 128:
                    dst = h_out[0:ow, ob * N + i * 128:ob * N + (i + 1) * 128]
                else:
                    dst = h_out[osl, nl]
                nc.scalar.activation(dst, tps[0:ow, :], AF.Identity)

        for i in range(NT + LAG):
            if i < NT:
                emit_topk_and_gather(i)
            j = i - LAG
            if j >= 0:
                emit_reduce(j)
                if post_reduce_cb is not None and j % 4 == 3:
                    post_reduce_cb(j // 4)

        if li < 3:
            nc.vector.tensor_copy(xr[0:O, :], h_out[0:O, :])
        if f"h{li}" in dbgt:
            if O > 128:
                nc.sync.dma_start(
                    out=dbgt[f"h{li}"][:, :].rearrange("(g p) n -> p g n", p=128),
                    in_=h_out.rearrange("p (g n) -> p g n", g=2))
            else:
                nc.sync.dma_start(out=dbgt[f"h{li}"][:, :], in_=h_out)


class _TailOverlap:
    """conv1d(512->1024)+BN+LReLU+global max/avg pool, emitted in 512-col
    chunks from inside L3's pipeline so it rides under the gather stream."""

    def __init__(self, nc, tpc, tpw, tpp, hts, wc, t4):
        self.nc = nc
        self.tpc = tpc
        self.tpw = tpw
        self.tpp = tpp
        self.hts = hts
        # 5 K-pieces (h0, h1, h2, h3a, h3b), each loaded at base partition 0
        self.wct = tpc.tile([128, 5 * 1024], FP, tag="wct")
        pieces = [(0, 64), (64, 128), (128, 256), (256, 384), (384, 512)]
        for pi, (r0, r1_) in enumerate(pieces):
            nc.sync.dma_start(
                out=self.wct[0:r1_ - r0, pi * 1024:(pi + 1) * 1024],
                in_=wc[r0:r1_, :])
        self.t4t = tpc.tile([128, 8], FP, tag="t4t")
        nc.sync.dma_start(out=self.t4t, in_=t4[:, :])
        # per-(mg, chunk) partial pools: [128, 8 mg * 4 chunks]
        self.zmax4 = tpc.tile([128, 32], FP, tag="zmax4")
        self.zsum4 = tpc.tile([128, 32], FP, tag="zsum4")

    def chunk(self, j):
        nc = self.nc
        cs = slice(j * 512, (j + 1) * 512)
        for mg in range(8):
            zps = self.tpp.tile([128, 512], FP, tag="zps")

            def wv(pi, kk):
                return self.wct[0:kk,
                                pi * 1024 + mg * 128:pi * 1024 + (mg + 1) * 128]

            nc.tensor.matmul(zps, wv(0, 64), self.hts[0][:, cs],
                             start=True, stop=False)
            nc.tensor.matmul(zps, wv(1, 64), self.hts[1][:, cs],
                             start=False, stop=False)
            nc.tensor.matmul(zps, wv(2, 128), self.hts[2][:, cs],
                             start=False, stop=False)
            nc.tensor.matmul(zps, wv(3, 128), self.hts[3][:, cs],
                             start=False, stop=False)
            nc.tensor.matmul(zps, wv(4, 128),
                             self.hts[3][:, N + cs.start:N + cs.stop],
                             start=False, stop=True)
            z4g = self.tpw.tile([128, 512], FP, tag="z4g")
            nc.scalar.activation(z4g, zps, AF.Identity,
                                 bias=self.t4t[:, mg:mg + 1])
            nc.vector.scalar_tensor_tensor(z4g, z4g, NEG, z4g,
                                           ALU.mult, ALU.max)
            nc.vector.tensor_reduce(self.zmax4[:, mg * 4 + j:mg * 4 + j + 1],
                                    z4g, mybir.AxisListType.X, ALU.max)
            nc.vector.tensor_reduce(self.zsum4[:, mg * 4 + j:mg * 4 + j + 1],
                                    z4g, mybir.AxisListType.X, ALU.add)

    def finish(self, tc, fdp, wl1, t5, wl2, t6, wl3, bl3, out, dbgt):
        nc = self.nc
        tpp = self.tpp
        with tc.tile_pool(name="Tfc", bufs=1) as tpc, \
             tc.tile_pool(name="Tfw", bufs=2) as tpw, \
             tc.tile_pool(name="Tfp", bufs=2, space="PSUM") as tfp:
            self._finish(tpc, tpw, tfp, fdp, wl1, t5, wl2, t6, wl3, bl3,
                         out, dbgt)

    def _finish(self, tpc, tpw, tpp, fdp, wl1, t5, wl2, t6, wl3, bl3,
                out, dbgt):
        nc = self.nc
        z16 = tpc.tile([128, 16], FP, tag="z16")
        for mg in range(8):
            nc.vector.tensor_reduce(z16[:, mg:mg + 1],
                                    self.zmax4[:, mg * 4:(mg + 1) * 4],
                                    mybir.AxisListType.X, ALU.max)
            nc.vector.tensor_reduce(z16[:, 8 + mg:9 + mg],
                                    self.zsum4[:, mg * 4:(mg + 1) * 4],
                                    mybir.AxisListType.X, ALU.add)
        if "zpool" in dbgt:
            nc.sync.dma_start(out=dbgt["zpool"][:, :], in_=z16)

        def fc(z_cols, w_T, M, bias_row, lrelu):
            # z_cols: [128, nk] tile (K-chunks as columns); w_T: [128*nk, M]
            nk = z_cols.shape[1]
            ps = tpp.tile([1, 512], FP, tag="fps")
            wt = tpc.tile([128, nk * M], FP, tag=f"w{M}")
            nc.sync.dma_start(out=wt.rearrange("p (a o) -> p a o", o=M),
                              in_=w_T[:, :].rearrange("(a p) o -> p a o", p=128))
            for kc in range(nk):
                nc.tensor.matmul(ps[0:1, 0:M], z_cols[:, kc:kc + 1],
                                 wt[:, kc * M:(kc + 1) * M],
                                 start=(kc == 0), stop=(kc == nk - 1))
            row = tpw.tile([1, M], FP, tag=f"row{M}")
            br = tpw.tile([1, M], FP, tag=f"br{M}")
            nc.sync.dma_start(out=br, in_=bias_row[:, :])
            nc.scalar.activation(row, ps[0:1, 0:M], AF.Identity)
            nc.vector.tensor_tensor(row, row, br, ALU.add)
            if lrelu:
                nc.vector.scalar_tensor_tensor(row, row, NEG, row,
                                               ALU.mult, ALU.max)
            return row

        r1 = fc(z16, wl1, 512, t5, True)                  # [1, 512]
        b1 = fdp.tile([512], FP, tag="fcb")
        nc.sync.dma_start(out=b1[:], in_=r1)
        z1c = tpc.tile([128, 4], FP, tag="z1c")
        nc.sync.dma_start(out=z1c, in_=b1.rearrange("(a p) -> p a", p=128))
        r2 = fc(z1c, wl2, 256, t6, True)                  # [1, 256]
        b2 = fdp.tile([512], FP, tag="fcb")
        nc.sync.dma_start(out=b2[0:256], in_=r2)
        z2c = tpc.tile([128, 2], FP, tag="z2c")
        nc.sync.dma_start(out=z2c,
                          in_=b2[0:256].rearrange("(a p) -> p a", p=128))
        r3 = fc(z2c, wl3, 40, bl3, False)                 # [1, 40]
        nc.sync.dma_start(out=out[:, :], in_=r3)


# ---------------------------------------------------------------------------
# Harness entry point: kernel(**inputs) -> np.ndarray [8, 40]
# ---------------------------------------------------------------------------
_NC_CACHE = {}


def _get_nc():
    if "nc" not in _NC_CACHE:
        _NC_CACHE["nc"] = build()
    return _NC_CACHE["nc"]


def kernel(**inputs):
    from concourse.bass_utils import run_bass_kernel_spmd

    nc = _get_nc()
    in_maps = [host_prep(inputs, core) for core in range(8)]
    res = run_bass_kernel_spmd(nc, in_maps, core_ids=list(range(8)))
    out = np.stack([r["out"][0] for r in res.results]).astype(np.float32)
    return out



# revision 19
# speedup vs baseline: 1.4796x; 1.0035x over previous
"""DGCNN Bass kernel for TRN2 — one sample per NeuronCore.

Math (per graph-conv layer, BN folded on host):
  u[n,m] = 2<x_n,x_m> - |x_m|^2   (row-wise top-20 == reference kNN)
  As = X^T (s*W1)^T [N,O];  Bs = X^T (s*(W2-W1))^T + t [N,O]
  h[n,o] = LReLU(max_{j in knn(n)} As[j,o] + Bs[n,o]);  X_next = h^T
Tail: conv1d(512->1024)+BN+LReLU, global max/avg pool, FC 2048->512->256->40.
"""
import numpy as np

import concourse.bass as bass
import concourse.bacc as bacc
import concourse.mybir as mybir
from concourse.tile import TileContext
from concourse.masks import make_identity

FP = mybir.dt.float32
AF = mybir.ActivationFunctionType
ALU = mybir.AluOpType
N = 2048
K = 20
NEG = 0.2
NT = N // 128  # 16 row tiles

LAYERS = [(3, 64), (64, 64), (64, 128), (128, 256)]  # (C_in, O)


def host_prep(inputs, core):
    """Per-core parameter dict from the full input dict (numpy arrays)."""
    d = {k: np.asarray(v, np.float32) for k, v in inputs.items()}
    p = {"x": np.ascontiguousarray(d["x"][core])}  # [3, N]
    for li, (c, o) in enumerate(LAYERS):
        w = d[f"w_gc{li}"]                      # [O, 2C]
        s = d[f"s{li}"]
        t = d[f"t{li}"]
        w1 = w[:, :c] * s[:, None]              # [O, C]
        w2 = (w[:, c:] - w[:, :c]) * s[:, None]
        p[f"w1_{li}"] = np.ascontiguousarray(w1.T)     # [C, O]
        p[f"w2_{li}"] = np.ascontiguousarray(w2.T)
        p[f"t_{li}"] = np.ascontiguousarray(
            np.broadcast_to(t[None, :], (128, o)))     # replicated bias
    p["w_c1d_T"] = np.ascontiguousarray((d["w_c1d"] * d["s4"][:, None]).T)  # [512,1024]
    p["t4"] = np.ascontiguousarray(d["t4"].reshape(8, 128).T)  # [128, 8] per-mg cols
    wl1 = d["w_l1"] * d["s5"][:, None]          # [512, 2048]
    wl1 = wl1.copy()
    wl1[:, 1024:] /= float(N)                   # fold avg-pool divisor
    p["w_l1_T"] = np.ascontiguousarray(wl1.T)   # [2048, 512]
    p["t5"] = np.ascontiguousarray(d["t5"][None, :])   # [1, 512]
    wl2 = d["w_l2"] * d["s6"][:, None]
    p["w_l2_T"] = np.ascontiguousarray(wl2.T)   # [512, 256]
    p["t6"] = np.ascontiguousarray(
        (d["s6"] * d["b_l2"] + d["t6"])[None, :])      # [1, 256]
    p["w_l3_T"] = np.ascontiguousarray(d["w_l3"].T)    # [256, 40]
    p["b_l3"] = np.ascontiguousarray(d["b_l3"][None, :])
    return p


def build(dbg=()):
    nc = bacc.Bacc(None, target_bir_lowering=False, num_swdge_queues=4)
    x_in = nc.declare_dram_parameter("x", [3, N], FP, isOutput=False)
    W1, W2, TB = {}, {}, {}
    for li, (c, o) in enumerate(LAYERS):
        W1[li] = nc.declare_dram_parameter(f"w1_{li}", [c, o], FP, isOutput=False)
        W2[li] = nc.declare_dram_parameter(f"w2_{li}", [c, o], FP, isOutput=False)
        TB[li] = nc.declare_dram_parameter(f"t_{li}", [128, o], FP, isOutput=False)
    wc = nc.declare_dram_parameter("w_c1d_T", [512, 1024], FP, isOutput=False)
    t4 = nc.declare_dram_parameter("t4", [128, 8], FP, isOutput=False)
    wl1 = nc.declare_dram_parameter("w_l1_T", [2048, 512], FP, isOutput=False)
    t5 = nc.declare_dram_parameter("t5", [1, 512], FP, isOutput=False)
    wl2 = nc.declare_dram_parameter("w_l2_T", [512, 256], FP, isOutput=False)
    t6 = nc.declare_dram_parameter("t6", [1, 256], FP, isOutput=False)
    wl3 = nc.declare_dram_parameter("w_l3_T", [256, 40], FP, isOutput=False)
    bl3 = nc.declare_dram_parameter("b_l3", [1, 40], FP, isOutput=False)
    out = nc.declare_dram_parameter("out", [1, 40], FP, isOutput=True)

    dbgt = {}
    for name in dbg:
        shp = {"u0": [128, N], "idx0": [128, 24],
               "h0": [64, N], "h1": [64, N], "h2": [128, N], "h3": [256, N],
               "z4": [128, N], "zpool": [128, 16]}[name]
        dbgt[name] = nc.declare_dram_parameter("dbg_" + name, shp, FP, isOutput=True)

    with TileContext(nc) as tc:
        with (
            tc.tile_pool(name="const", bufs=1) as cp,
            tc.tile_pool(name="dram", bufs=2, space="DRAM") as dp,
            tc.tile_pool(name="x", bufs=1) as xp,
        ):
            ident = cp.tile([128, 128], FP)
            make_identity(nc, ident)
            xl = xp.tile([128, N], FP)   # [2X; ones(row C) for C<128]
            xr = xp.tile([128, N], FP)   # [X; -sq(row C) for C<128]
            xa1 = xp.tile([1, N], FP)    # ones row (L4)
            xan = xp.tile([1, N], FP)    # -sq row (L4)
            hts = []
            for li in range(4):
                o_l = LAYERS[li][1]
                h_t = xp.tile([min(o_l, 128), (o_l // 128 or 1) * N], FP,
                              tag=f"h{li}")
                hts.append(h_t)
            nc.sync.dma_start(out=xr[0:3, :], in_=x_in[:, :])

            for li in range(3):
                C, O = LAYERS[li]
                _layer(nc, tc, dp, li, C, O, xl, xr, xa1, xan, ident,
                       W1[li], W2[li], TB[li], hts[li], dbgt)

            # L3 with the conv1d+pool tail overlapped into its gather window
            with (
                tc.tile_pool(name="Tc", bufs=1) as tpc,
                tc.tile_pool(name="Tw", bufs=2) as tpw,
                tc.tile_pool(name="Tp", bufs=2, space="PSUM") as tpp,
                tc.tile_pool(name="Td", bufs=2, space="DRAM") as tdp,
            ):
                tail = _TailOverlap(nc, tpc, tpw, tpp, hts, wc, t4)
                C, O = LAYERS[3]
                _layer(nc, tc, dp, 3, C, O, xl, xr, xa1, xan, ident,
                       W1[3], W2[3], TB[3], hts[3], dbgt,
                       post_reduce_cb=tail.chunk)
                tail.finish(tc, tdp, wl1, t5, wl2, t6, wl3, bl3, out, dbgt)
    nc.finalize()
    return nc


def _layer(nc, tc, dp, li, C, O, xl, xr, xa1, xan, ident, w1, w2, tbias,
           h_out, dbgt, post_reduce_cb=None):
    Ka = C + 1
    use_self = li < 3          # skip k=0 gather via SBUF As (SBUF-tight on L3)
    k0 = 1 if use_self else 0
    as_dram = dp.tile([N, 256], FP, tag="as_dram")
    with (
        tc.tile_pool(name=f"L{li}", bufs=3) as pool,
        tc.tile_pool(name=f"L{li}u", bufs=1) as pu,
        tc.tile_pool(name=f"L{li}ud", bufs=2 if li < 3 else 1) as pud,
        tc.tile_pool(name=f"L{li}c", bufs=1) as pc,
        tc.tile_pool(name=f"L{li}g", bufs=2 if li >= 2 else 3) as pg,
        tc.tile_pool(name=f"L{li}up", bufs=1, space="PSUM") as ppu,
        tc.tile_pool(name=f"L{li}sp", bufs=3 if li < 3 else 2,
                     space="PSUM") as pps,
    ):
        # ---- augmented X ----
        nc.vector.tensor_scalar(xl[0:C, :], xr[0:C, :], 2.0, None, ALU.mult)
        nc.vector.memset(xa1, 1.0)
        xsq = pc.tile([C, N], FP, tag="xsq")
        nc.scalar.activation(xsq, xr[0:C, :], AF.Square)
        ones_c = pc.tile([C, 1], FP, tag="ones_c")
        nc.vector.memset(ones_c, 1.0)
        for j in range(4):
            cs = slice(j * 512, (j + 1) * 512)
            sq_ps = pps.tile([1, 512], FP, tag="sps")
            nc.tensor.matmul(sq_ps, ones_c[0:C, :], xsq[0:C, cs],
                             start=True, stop=True)
            nc.scalar.activation(xan[:, cs], sq_ps, AF.Identity, scale=-1.0)
        if C < 128:
            # place aug rows at partition C (DMA: no base-partition limits)
            nc.sync.dma_start(out=xl[C:C + 1, :], in_=xa1)
            nc.sync.dma_start(out=xr[C:C + 1, :], in_=xan)

        # ---- weights ----
        w1t = pc.tile([C, O], FP, tag="w1t")
        w2t = pc.tile([C, O], FP, tag="w2t")
        tb = pc.tile([128, O], FP, tag="tb")
        nc.sync.dma_start(out=w1t, in_=w1[:, :])
        nc.sync.dma_start(out=w2t, in_=w2[:, :])
        nc.sync.dma_start(out=tb, in_=tbias[:, :])

        if use_self:
            as_all = pc.tile([128, NT * O], FP, tag="as_all")
        else:
            as_all = None
        bs_all = pc.tile([128, NT * O], FP, tag="bs_all")
        idx_all = pc.tile([128, NT * 24], mybir.dt.uint32, tag="idx_all")
        as_view = as_dram[:, :].rearrange("n o -> (n o)")[0:N * O]            .rearrange("(n o) -> n o", o=O)

        # ======== pass A1: As/Bs for all tiles (As -> DRAM + SBUF) ========
        for i in range(NT):
            nl = slice(i * 128, (i + 1) * 128)
            asps = pps.tile([128, O], FP, tag="sps")
            nc.tensor.matmul(asps, xr[0:C, nl], w1t[0:C, :], start=True, stop=True)
            if use_self:
                as_sb = as_all[:, i * O:(i + 1) * O]
            else:
                as_sb = pool.tile([128, O], FP, tag="as_sb")
            nc.scalar.activation(as_sb, asps, AF.Identity)
            nc.sync.dma_start(out=as_view[nl, :], in_=as_sb)
            bsps = pps.tile([128, O], FP, tag="sps")
            nc.tensor.matmul(bsps, xr[0:C, nl], w2t[0:C, :], start=True, stop=True)
            nc.vector.tensor_tensor(bs_all[:, i * O:(i + 1) * O], bsps, tb, ALU.add)

        # ==== pass A2+B fused, software-pipelined with LAG ====
        # topk_i (Vector) interleaves with reduce_{i-LAG} so the gather
        # pool recycles while GpSimd streams INDIRECT1Ds continuously.
        LAG = 2
        gaths = [None] * NT

        def emit_topk_and_gather(i):
            nl = slice(i * 128, (i + 1) * 128)
            ups = ppu.tile([128, N], FP, tag="ups")
            for j in range(4):
                cs = slice(j * 512, (j + 1) * 512)
                if Ka <= 128:
                    nc.tensor.matmul(ups[:, cs], xl[0:Ka, nl], xr[0:Ka, cs],
                                     start=True, stop=True)
                else:
                    nc.tensor.matmul(ups[:, cs], xl[0:C, nl], xr[0:C, cs],
                                     start=True, stop=False)
                    nc.tensor.matmul(ups[:, cs], xa1[:, nl], xan[:, cs],
                                     start=False, stop=True)
            u = pud.tile([128, N], FP, tag="u")
            nc.scalar.activation(u, ups, AF.Identity)
            if "u0" in dbgt and li == 0 and i == 0:
                nc.sync.dma_start(out=dbgt["u0"][:, :], in_=u)

            vals = pool.tile([128, 24], FP, tag="vals")
            idxu = idx_all[:, i * 24:(i + 1) * 24]
            u2 = pu.tile([128, N], FP, tag="u2")
            u3 = pu.tile([128, N], FP, tag="u3")
            nc.vector.max(out=vals[:, 0:8], in_=u)
            nc.vector.max_index(out=idxu[:, 0:8], in_max=vals[:, 0:8], in_values=u)
            nc.vector.match_replace(out=u2, in_to_replace=vals[:, 0:8],
                                    in_values=u, imm_value=-1e30)
            nc.vector.max(out=vals[:, 8:16], in_=u2)
            nc.vector.max_index(out=idxu[:, 8:16], in_max=vals[:, 8:16],
                                in_values=u2)
            nc.vector.match_replace(out=u3, in_to_replace=vals[:, 8:16],
                                    in_values=u2, imm_value=-1e30)
            nc.vector.max(out=vals[:, 16:24], in_=u3)
            nc.vector.max_index(out=idxu[:, 16:24], in_max=vals[:, 16:24],
                                in_values=u3)
            if "idx0" in dbgt and li == 0 and i == 0:
                fi = pool.tile([128, 24], FP, tag="fi")
                nc.vector.tensor_copy(fi, idxu)
                nc.sync.dma_start(out=dbgt["idx0"][:, :], in_=fi)

            # gather k=k0..19 (k=0 is the node itself, in as_all when kept)
            gath = pg.tile([128, K - k0, O], FP, tag="gath")
            gaths[i] = gath
            for kk in range(k0, K):
                nc.gpsimd.indirect_dma_start(
                    out=gath[:, kk - k0, :], out_offset=None,
                    in_=as_view[0:N, 0:O],
                    in_offset=bass.IndirectOffsetOnAxis(
                        ap=idx_all[:, i * 24 + kk:i * 24 + kk + 1], axis=0))

        def emit_reduce(i):
            nl = slice(i * 128, (i + 1) * 128)
            g = gaths[i]
            nk = K - k0
            # contiguous pairwise-max tree over k (in place inside gath)
            tt = nc.vector.tensor_tensor
            tt(g[:, 0:8, :], g[:, 0:8, :], g[:, 8:16, :], ALU.max)
            tt(g[:, 0:4, :], g[:, 0:4, :], g[:, 4:8, :], ALU.max)
            if nk == 20:
                tt(g[:, 0:4, :], g[:, 0:4, :], g[:, 16:20, :], ALU.max)
                tt(g[:, 0:2, :], g[:, 0:2, :], g[:, 2:4, :], ALU.max)
            else:  # nk == 19: fold leftover k=16..18
                tt(g[:, 0:2, :], g[:, 0:2, :], g[:, 2:4, :], ALU.max)
                tt(g[:, 0:2, :], g[:, 0:2, :], g[:, 16:18, :], ALU.max)
                tt(g[:, 0:1, :], g[:, 0:1, :], g[:, 18:19, :], ALU.max)
            m = pool.tile([128, O], FP, tag="m")
            tt(m, g[:, 0, :], g[:, 1, :], ALU.max)
            if use_self:
                tt(m, m, as_all[:, i * O:(i + 1) * O], ALU.max)
            tt(m, m, bs_all[:, i * O:(i + 1) * O], ALU.add)
            hl = pool.tile([128, O], FP, tag="hl")
            nc.vector.scalar_tensor_tensor(hl, m, NEG, m, ALU.mult, ALU.max)
            for ob in range((O + 127) // 128):
                osl = slice(ob * 128, min((ob + 1) * 128, O))
                ow = osl.stop - osl.start
                tps = pps.tile([128, 128], FP, tag="sps")
                nc.tensor.transpose(tps[0:ow, :], hl[:, osl], ident)
                if O > 128:
                    dst = h_out[0:ow, ob * N + i * 128:ob * N + (i + 1) * 128]
                else:
                    dst = h_out[osl, nl]
                nc.scalar.activation(dst, tps[0:ow, :], AF.Identity)

        for i in range(NT + LAG):
            if i < NT:
                emit_topk_and_gather(i)
            j = i - LAG
            if j >= 0:
                emit_reduce(j)
                if post_reduce_cb is not None and j % 4 == 3:
                    post_reduce_cb(j // 4)

        if li < 3:
            nc.vector.tensor_copy(xr[0:O, :], h_out[0:O, :])
        if f"h{li}" in dbgt:
            if O > 128:
                nc.sync.dma_start(
                    out=dbgt[f"h{li}"][:, :].rearrange("(g p) n -> p g n", p=128),
                    in_=h_out.rearrange("p (g n) -> p g n", g=2))
            else:
                nc.sync.dma_start(out=dbgt[f"h{li}"][:, :], in_=h_out)


class _TailOverlap:
    """conv1d(512->1024)+BN+LReLU+global max/avg pool, emitted in 512-col
    chunks from inside L3's pipeline so it rides under the gather stream."""

    def __init__(self, nc, tpc, tpw, tpp, hts, wc, t4):
        self.nc = nc
        self.tpc = tpc
        self.tpw = tpw
        self.tpp = tpp
        self.hts = hts
        # 5 K-pieces (h0, h1, h2, h3a, h3b), each loaded at base partition 0
        self.wct = tpc.tile([128, 5 * 1024], FP, tag="wct")
        pieces = [(0, 64), (64, 128), (128, 256), (256, 384), (384, 512)]
        for pi, (r0, r1_) in enumerate(pieces):
            nc.sync.dma_start(
                out=self.wct[0:r1_ - r0, pi * 1024:(pi + 1) * 1024],
                in_=wc[r0:r1_, :])
        self.t4t = tpc.tile([128, 8], FP, tag="t4t")
        nc.sync.dma_start(out=self.t4t, in_=t4[:, :])
        # per-(mg, chunk) partial pools: [128, 8 mg * 4 chunks]
        self.zmax4 = tpc.tile([128, 32], FP, tag="zmax4")
        self.zsum4 = tpc.tile([128, 32], FP, tag="zsum4")

    def chunk(self, j):
        nc = self.nc
        cs = slice(j * 512, (j + 1) * 512)
        for mg in range(8):
            zps = self.tpp.tile([128, 512], FP, tag="zps")

            def wv(pi, kk):
                return self.wct[0:kk,
                                pi * 1024 + mg * 128:pi * 1024 + (mg + 1) * 128]

            nc.tensor.matmul(zps, wv(0, 64), self.hts[0][:, cs],
                             start=True, stop=False)
            nc.tensor.matmul(zps, wv(1, 64), self.hts[1][:, cs],
                             start=False, stop=False)
            nc.tensor.matmul(zps, wv(2, 128), self.hts[2][:, cs],
                             start=False, stop=False)
            nc.tensor.matmul(zps, wv(3, 128), self.hts[3][:, cs],
                             start=False, stop=False)
            nc.tensor.matmul(zps, wv(4, 128),
                             self.hts[3][:, N + cs.start:N + cs.stop],
                             start=False, stop=True)
            z4g = self.tpw.tile([128, 512], FP, tag="z4g")
            nc.scalar.activation(z4g, zps, AF.Identity,
                                 bias=self.t4t[:, mg:mg + 1])
            nc.vector.scalar_tensor_tensor(z4g, z4g, NEG, z4g,
                                           ALU.mult, ALU.max)
            nc.vector.tensor_reduce(self.zmax4[:, mg * 4 + j:mg * 4 + j + 1],
                                    z4g, mybir.AxisListType.X, ALU.max)
            nc.vector.tensor_reduce(self.zsum4[:, mg * 4 + j:mg * 4 + j + 1],
                                    z4g, mybir.AxisListType.X, ALU.add)

    def finish(self, tc, fdp, wl1, t5, wl2, t6, wl3, bl3, out, dbgt):
        nc = self.nc
        tpp = self.tpp
        with tc.tile_pool(name="Tfc", bufs=1) as tpc, \
             tc.tile_pool(name="Tfw", bufs=2) as tpw, \
             tc.tile_pool(name="Tfp", bufs=2, space="PSUM") as tfp:
            self._finish(tpc, tpw, tfp, fdp, wl1, t5, wl2, t6, wl3, bl3,
                         out, dbgt)

    def _finish(self, tpc, tpw, tpp, fdp, wl1, t5, wl2, t6, wl3, bl3,
                out, dbgt):
        nc = self.nc
        z16 = tpc.tile([128, 16], FP, tag="z16")
        for mg in range(8):
            nc.vector.tensor_reduce(z16[:, mg:mg + 1],
                                    self.zmax4[:, mg * 4:(mg + 1) * 4],
                                    mybir.AxisListType.X, ALU.max)
            nc.vector.tensor_reduce(z16[:, 8 + mg:9 + mg],
                                    self.zsum4[:, mg * 4:(mg + 1) * 4],
                                    mybir.AxisListType.X, ALU.add)
        if "zpool" in dbgt:
            nc.sync.dma_start(out=dbgt["zpool"][:, :], in_=z16)

        def fc(z_cols, w_T, M, bias_row, lrelu):
            # z_cols: [128, nk] tile (K-chunks as columns); w_T: [128*nk, M]
            nk = z_cols.shape[1]
            ps = tpp.tile([1, 512], FP, tag="fps")
            wt = tpc.tile([128, nk * M], FP, tag=f"w{M}")
            nc.sync.dma_start(out=wt.rearrange("p (a o) -> p a o", o=M),
                              in_=w_T[:, :].rearrange("(a p) o -> p a o", p=128))
            for kc in range(nk):
                nc.tensor.matmul(ps[0:1, 0:M], z_cols[:, kc:kc + 1],
                                 wt[:, kc * M:(kc + 1) * M],
                                 start=(kc == 0), stop=(kc == nk - 1))
            row = tpw.tile([1, M], FP, tag=f"row{M}")
            br = tpw.tile([1, M], FP, tag=f"br{M}")
            nc.sync.dma_start(out=br, in_=bias_row[:, :])
            nc.scalar.activation(row, ps[0:1, 0:M], AF.Identity)
            nc.vector.tensor_tensor(row, row, br, ALU.add)
            if lrelu:
                nc.vector.scalar_tensor_tensor(row, row, NEG, row,
                                               ALU.mult, ALU.max)
            return row

        r1 = fc(z16, wl1, 512, t5, True)                  # [1, 512]
        b1 = fdp.tile([512], FP, tag="fcb")
        nc.sync.dma_start(out=b1[:], in_=r1)
        z1c = tpc.tile([128, 4], FP, tag="z1c")
        nc.sync.dma_start(out=z1c, in_=b1.rearrange("(a p) -> p a", p=128))
        r2 = fc(z1c, wl2, 256, t6, True)                  # [1, 256]
        b2 = fdp.tile([512], FP, tag="fcb")
        nc.sync.dma_start(out=b2[0:256], in_=r2)
        z2c = tpc.tile([128, 2], FP, tag="z2c")
        nc.sync.dma_start(out=z2c,
                          in_=b2[0:256].rearrange("(a p) -> p a", p=128))
        r3 = fc(z2c, wl3, 40, bl3, False)                 # [1, 40]
        nc.sync.dma_start(out=out[:, :], in_=r3)


# ---------------------------------------------------------------------------
# Harness entry point: kernel(**inputs) -> np.ndarray [8, 40]
# ---------------------------------------------------------------------------
_NC_CACHE = {}


def _get_nc():
    if "nc" not in _NC_CACHE:
        _NC_CACHE["nc"] = build()
    return _NC_CACHE["nc"]


def kernel(**inputs):
    from concourse.bass_utils import run_bass_kernel_spmd

    nc = _get_nc()
    in_maps = [host_prep(inputs, core) for core in range(8)]
    res = run_bass_kernel_spmd(nc, in_maps, core_ids=list(range(8)))
    out = np.stack([r["out"][0] for r in res.results]).astype(np.float32)
    return out

